# revision 21
# baseline (speedup 1.0000x reference)
"""DCNv2 deformable ROI pooling on 8 Trainium2 NeuronCores.

Strategy: per-bin the 4x4 bilinear sample grid is separable (y outer-product
x), so each ROI's pooled output reduces to one small accumulated matmul
    out[49 bins, 256 ch] = M[49, K] @ PatchFlat[K, 256]
where K = R*L is a flattened feature-map patch window covering the ROI's
samples and M = alpha (x) beta is built from host-precomputed per-axis
interpolation weights.  ROIs (dim 0) are sharded across the 8 cores; the
channels-last feature map is replicated.

Patch windows come in size classes (R, L) in {16,24}^2 picked per ROI from
its actual sample span; per-class slot counts are chosen identically for
every core (round-robin distribution + dummy padding) so a single NEFF runs
SPMD on all 8 cores.  Per-ROI patch addresses are runtime data (register
offset DMA).

Patch chunk layout for class (R, L) with G col-groups (G*R = Q partitions,
nk = L/G chunks): partition p = r*G + s holds pixels (row r, col s*nk + k)
for chunk k, giving a plain [Q, nk*C]-destination DMA whose source is R
contiguous L*C-element row segments.
"""

import numpy as np

import concourse.bass as bass
import concourse.mybir as mybir
import concourse.tile as tile
from concourse import bacc
import concourse.bass_utils as bass_utils

B, C, H, W = 4, 256, 128, 128
N_ROIS = 512
P = 7
PP = P * P
SCALE = np.float32(0.0625)
RATIO = 4
GAMMA = np.float32(0.1)
N_CORES = 8

# Patch size classes.  G col-groups per row: partition p = r*G + s holds
# pixels (row r, col s*nk + k) for chunk k; Q = G*R partitions, nk = L/G
# chunks.  G is chosen per R to maximize partition fill (fewer, fuller
# matmul chunks); L options per R must be multiples of G.
R_SPECS = [
    (12, 8, (8, 16, 24, 32)),
    (16, 8, (8, 16, 24, 32)),
    (24, 4, (8, 12, 16, 20, 24, 28, 32)),
    (32, 4, (12, 16, 20, 24, 28, 32)),
]
CLASS_SPECS = {}
CLASS_ORDER = []
for _r, _g, _lopts in R_SPECS:
    for _l in _lopts:
        CLASS_SPECS[(_r, _l)] = (_g, _g * _r, _l // _g)
        CLASS_ORDER.append((_r, _l))
# biggest patches first: fill the DMA pipe early, drain small slots last
CLASS_ORDER.sort(key=lambda rl: -rl[0] * rl[1])

# Matmul precision mode:
#   "f32"  - exact float32 matmuls (4 cycles/row on PE)
#   "bf16" - inputs/weights rounded to bfloat16 (1 cycle/row, ~4e-3 rel err)
#   "pair" - bfloat16 hi/lo split of both operands, 3 matmuls per chunk with
#            exact bf16xbf16 products accumulated in fp32 (~1e-5 rel err,
#            3 cycles/row net)
MM_DTYPE = "bf16"

_f32 = np.float32


def _prep(rois, offset):
    """Dense per-axis interpolation weights + per-ROI sample bounds.

    Returns (bidx, ymin, ymax, xmin, xmax, alpha_d[N,PP,H], beta_d[N,PP,W]).
    """
    n = rois.shape[0]
    bidx = rois[:, 0].astype(np.int32)
    x1 = rois[:, 1] * SCALE - _f32(0.5)
    y1 = rois[:, 2] * SCALE - _f32(0.5)
    x2 = rois[:, 3] * SCALE - _f32(0.5)
    y2 = rois[:, 4] * SCALE - _f32(0.5)
    rw = np.maximum(x2 - x1, _f32(1.0))
    rh = np.maximum(y2 - y1, _f32(1.0))
    bw = rw / _f32(P)
    bh = rh / _f32(P)
    off = offset.reshape(n, 2, P, P).astype(np.float32)
    off_x = GAMMA * rw[:, None, None] * off[:, 0]
    off_y = GAMMA * rh[:, None, None] * off[:, 1]
    ph = np.arange(P, dtype=np.float32)
    s = ((np.arange(RATIO, dtype=np.float32) + _f32(0.5)) / _f32(RATIO))
    # mirror reference.py op order exactly (float32)
    ybase = y1[:, None, None] + ph[None, :, None] * bh[:, None, None] + off_y
    xbase = x1[:, None, None] + ph[None, None, :] * bw[:, None, None] + off_x
    ys = ybase[..., None] + s[None, None, None, :] * bh[:, None, None, None]
    xs = xbase[..., None] + s[None, None, None, :] * bw[:, None, None, None]
    vy = (ys > -1.0) & (ys < H)
    vx = (xs > -1.0) & (xs < W)
    yc = np.clip(ys, _f32(0.0), _f32(H - 1))
    xc = np.clip(xs, _f32(0.0), _f32(W - 1))
    y0 = np.floor(yc).astype(np.int32)
    x0 = np.floor(xc).astype(np.int32)
    y1i = np.minimum(y0 + 1, H - 1)
    x1i = np.minimum(x0 + 1, W - 1)
    ly = (yc - y0).astype(np.float32)
    lx = (xc - x0).astype(np.float32)
    hy = _f32(1.0) - ly
    hx = _f32(1.0) - lx

    npp = n * PP
    alpha_d = np.zeros((npp, H), np.float32)
    beta_d = np.zeros((npp, W), np.float32)
    rows = np.repeat(np.arange(npp), RATIO)
    inv = _f32(1.0 / RATIO)
    np.add.at(alpha_d, (rows, y0.reshape(npp, RATIO).ravel()),
              (np.where(vy, hy, 0).reshape(npp, RATIO) * inv).ravel())
    np.add.at(alpha_d, (rows, y1i.reshape(npp, RATIO).ravel()),
              (np.where(vy, ly, 0).reshape(npp, RATIO) * inv).ravel())
    np.add.at(beta_d, (rows, x0.reshape(npp, RATIO).ravel()),
              (np.where(vx, hx, 0).reshape(npp, RATIO) * inv).ravel())
    np.add.at(beta_d, (rows, x1i.reshape(npp, RATIO).ravel()),
              (np.where(vx, lx, 0).reshape(npp, RATIO) * inv).ravel())

    ymin = np.minimum(y0.reshape(n, -1).min(axis=1), 127)
    ymax = np.minimum(y1i.reshape(n, -1).max(axis=1), 127)
    xmin = np.minimum(x0.reshape(n, -1).min(axis=1), 127)
    xmax = np.minimum(x1i.reshape(n, -1).max(axis=1), 127)
    return (bidx, ymin, ymax, xmin, xmax,
            alpha_d.reshape(n, PP, H), beta_d.reshape(n, PP, W))


def _mt_block(alpha_w, beta_w, R, L):
    """[PP, R] x [PP, L] weights -> device MT block [Q, nk*PP]."""
    G, Q, nk = CLASS_SPECS[(R, L)]
    p = np.arange(Q)
    a = alpha_w[:, p // G]                        # [PP, Q]
    l_idx = (p[:, None] % G) * nk + np.arange(nk)[None, :]   # [Q, nk]
    b = beta_w[:, l_idx]                          # [PP, Q, nk]
    mt = a.T[:, None, :] * b.transpose(1, 2, 0)   # [Q, nk, PP]
    return mt.reshape(Q, nk * PP).astype(np.float32)


def _layout_meta(layout):
    """Free-dim offsets of each slot's MT block in the resident SBUF tile,
    total free size, and the <=4 load-chunk split points (slot-aligned)."""
    pair_f = 2 if MM_DTYPE == "pair" else 1
    fo = []
    f = 0
    for rl in layout:
        G, Q, nk = CLASS_SPECS[rl]
        fo.append(f)
        f += nk * PP * pair_f
    bounds = fo + [f]
    n_chunks = 4
    splits = [0]
    for j in range(1, n_chunks):
        target = f * j // n_chunks
        splits.append(min(bounds, key=lambda b_: abs(b_ - target)))
    splits.append(f)
    splits = sorted(set(splits))
    return fo, f, splits


OUT_GROUP = 16  # slots per packed output flush
PATCH_BUFS = 16
PSUM_BUFS = 8   # [PP, 2C] f32 pair tiles, one PSUM bank each
SP_NUM, SP_DEN = 8, 16   # patch DMA share on the SP ring (rest on ACT)
OFF_CHUNK = 2   # offsets per batched register load


def _sp_flags(gs):
    """Per-slot ring assignment within a group: evenly spread SP_NUM/SP_DEN
    of the slots onto the SP ring, the rest onto ACT."""
    n_sp = (gs * SP_NUM + SP_DEN - 1) // SP_DEN
    return [(j + 1) * n_sp // gs > j * n_sp // gs for j in range(gs)]


def _po_positions(n_slots):
    """slot -> index in the po tensor (group-major, SP slots before ACT)."""
    po_pos = {}
    p = 0
    for g0 in range(0, n_slots, OUT_GROUP):
        gs = min(OUT_GROUP, n_slots - g0)
        flags = _sp_flags(gs)
        for i in [g0 + j for j in range(gs) if flags[j]] + \
                 [g0 + j for j in range(gs) if not flags[j]]:
            po_pos[i] = p
            p += 1
    return po_pos


_NC_CACHE = {}


def _build_kernel(layout):
    """layout: tuple of (R, L) per slot, identical on every core."""
    key = (tuple(layout), MM_DTYPE)
    if key in _NC_CACHE:
        return _NC_CACHE[key]
    n_slots = len(layout)
    fo, mt_free, splits = _layout_meta(layout)
    pair = MM_DTYPE == "pair"
    data_dt = (mybir.dt.bfloat16 if MM_DTYPE in ("bf16", "pair")
               else mybir.dt.float32)
    mm_dt = {"f32": mybir.dt.float32, "f32r": mybir.dt.float32r,
             "bf16": mybir.dt.bfloat16, "pair": mybir.dt.bfloat16}[MM_DTYPE]
    cpp = 2 if pair else 1  # channel planes per pixel in xt / patch

    nc = bacc.Bacc("TRN2", target_bir_lowering=False, debug=False,
                   num_devices=N_CORES)
    xt_shape = [B, H, W, cpp * C] if pair else [B, H, W, C]
    xt = nc.dram_tensor("xt", xt_shape, data_dt,
                        kind="ExternalInput").ap()
    mt = nc.dram_tensor("mt", [128, mt_free], data_dt,
                        kind="ExternalInput").ap()
    po = nc.dram_tensor("po", [1, n_slots], mybir.dt.int32,
                        kind="ExternalInput").ap()
    n_groups = -(-n_slots // OUT_GROUP)
    # group-major output: out[g, b, s*C + c] holds slot g*OUT_GROUP+s
    out = nc.dram_tensor("out", [n_groups, PP, OUT_GROUP * C],
                         mybir.dt.float32, kind="ExternalOutput").ap()

    groups = [(g, min(OUT_GROUP, n_slots - g)) for g in range(0, n_slots, OUT_GROUP)]
    n_groups_ = len(groups)
    bounds = fo + [mt_free]
    cc = cpp * C
    pair_f = 2 if pair else 1

    # per-group ring split and po issue-order positions (host mirrors this)
    g_ring = []
    for g0, gs in groups:
        flags = _sp_flags(gs)
        g_ring.append(([g0 + j for j in range(gs) if flags[j]],
                       [g0 + j for j in range(gs) if not flags[j]]))
    po_pos = _po_positions(n_slots)
    gw_max = max(bounds[g0 + gs] - bounds[g0] for g0, gs in groups)

    def _max_off(i):
        R, L = layout[i]
        return (((B - 1) * H + (H - R)) * W + (W - L)) * cc

    with tile.TileContext(nc) as tc:
        with (
            tc.tile_pool(name="offp", bufs=1) as offp,
            tc.tile_pool(name="mtp", bufs=4) as mtp,
            tc.tile_pool(name="patchp", bufs=PATCH_BUFS) as patchp,
            tc.tile_pool(name="outp", bufs=3) as outp,
            tc.tile_pool(name="psump", bufs=PSUM_BUFS, space="PSUM") as psump,
        ):
            offs = offp.tile([1, n_slots], mybir.dt.int32)
            nc.sync.dma_start(offs[:, :], po[:, :])

            mt_tiles = [None] * n_groups_
            off_vals = [None] * n_groups_

            def load_mt(gi):
                g0, gs = groups[gi]
                w = bounds[g0 + gs] - bounds[g0]
                t = mtp.tile([128, gw_max], data_dt, tag="mt")
                eng = nc.sync if gi % 2 == 0 else nc.scalar
                eng.dma_start(t[:, 0:w], mt[:, bounds[g0]:bounds[g0 + gs]])
                mt_tiles[gi] = t

            def off_thunks(gi):
                """One thunk per OFF_CHUNK register batch, to be emitted
                interleaved with patch DMAs so the ~480ns/reg load latency
                never blocks a run of patch issues."""
                off_vals[gi] = {}
                thunks = []
                for eng, lst in ((mybir.EngineType.SP, g_ring[gi][0]),
                                 (mybir.EngineType.Activation, g_ring[gi][1])):
                    if not lst:
                        continue
                    base = po_pos[lst[0]]
                    for c0 in range(0, len(lst), OFF_CHUNK):
                        chunk = lst[c0:c0 + OFF_CHUNK]

                        def th(eng=eng, base=base, c0=c0, chunk=chunk, gi=gi):
                            _, vs = nc.values_load_multi_w_load_instructions(
                                offs[0:1, base + c0:base + c0 + len(chunk)],
                                engines=[eng],
                                min_val=0,
                                max_val=max(_max_off(i) for i in chunk),
                                skip_runtime_bounds_check=True)
                            off_vals[gi].update(zip(chunk, vs))
                        thunks.append(th)
                return thunks

            for th in off_thunks(0):
                th()
            load_mt(0)
            if n_groups_ > 1:
                for th in off_thunks(1):
                    th()
                load_mt(1)

            pending = None
            for gi, (g0, gs) in enumerate(groups):
                osb = outp.tile([PP, OUT_GROUP * C], mybir.dt.float32,
                                tag="osb")
                if gs < OUT_GROUP:
                    nc.vector.memset(osb[:, gs * C:], 0.0)
                mt_sb = mt_tiles[gi]
                vals = off_vals[gi]
                sp_set = set(g_ring[gi][0])
                mb = bounds[g0]
                # work to interleave between this group's pairs: flush of
                # the previous group, then prefetch of group gi+2
                inter = []
                if pending is not None:
                    posb, pg = pending

                    def fl1(posb=posb, pg=pg):
                        nc.sync.dma_start(out[pg][0:25], posb[0:25, :])

                    def fl2(posb=posb, pg=pg):
                        nc.scalar.dma_start(out[pg][25:PP], posb[25:PP, :])
                    inter += [fl1, fl2]
                if gi + 2 < n_groups_:
                    inter += off_thunks(gi + 2)
                    inter.append(lambda gi=gi: load_mt(gi + 2))
                for jp in range(0, gs, 2):
                    pr = [g0 + jp] + ([g0 + jp + 1] if jp + 1 < gs else [])
                    ps = psump.tile([PP, 2 * C], mybir.dt.float32,
                                    space="PSUM")
                    for h, i in enumerate(pr):
                        R, L = layout[i]
                        G, Q, nk = CLASS_SPECS[(R, L)]
                        patch = patchp.tile([Q, nk * cc], data_dt,
                                            tag="patch")
                        issuer = nc.sync if i in sp_set else nc.scalar
                        src = bass.AP(xt.tensor, vals[i],
                                      [[W * cc, R], [1, L * cc]])
                        issuer.dma_start(patch[:, :], src)
                        pv = ps[:, h * C:(h + 1) * C]
                        f0 = fo[i] - mb
                        if not pair:
                            for k in range(nk):
                                lhsT = mt_sb[0:Q,
                                             f0 + k * PP:f0 + (k + 1) * PP]
                                rhs = patch[:, k * C:(k + 1) * C]
                                if mm_dt != data_dt:
                                    lhsT = lhsT.bitcast(mm_dt)
                                    rhs = rhs.bitcast(mm_dt)
                                nc.tensor.matmul(
                                    pv, lhsT=lhsT, rhs=rhs,
                                    start=(k == 0), stop=(k == nk - 1))
                        else:
                            # hi/lo pair: out = Mhi@Xhi + Mlo@Xhi + Mhi@Xlo
                            for k in range(nk):
                                mhi = mt_sb[0:Q, f0 + 2 * k * PP:
                                            f0 + (2 * k + 1) * PP]
                                mlo = mt_sb[0:Q, f0 + (2 * k + 1) * PP:
                                            f0 + (2 * k + 2) * PP]
                                xhi = patch[:, 2 * k * C:(2 * k + 1) * C]
                                xlo = patch[:, (2 * k + 1) * C:(2 * k + 2) * C]
                                nc.tensor.matmul(pv, lhsT=mhi, rhs=xhi,
                                                 start=(k == 0), stop=False)
                                nc.tensor.matmul(pv, lhsT=mlo, rhs=xhi,
                                                 start=False, stop=False)
                                nc.tensor.matmul(pv, lhsT=mhi, rhs=xlo,
                                                 start=False,
                                                 stop=(k == nk - 1))
                    nc.vector.tensor_copy(
                        osb[:, jp * C:(jp + len(pr)) * C],
                        ps[:, 0:len(pr) * C])
                    if gi == n_groups_ - 1:
                        # last group: flush each pair as soon as it's
                        # copied, so the final drain is one pair, not
                        # the whole group
                        eng = nc.sync if (jp // 2) % 2 == 0 else nc.scalar
                        w = len(pr) * C if jp + 2 < gs else \
                            (OUT_GROUP - jp) * C
                        eng.dma_start(out[gi][:, jp * C:jp * C + w],
                                      osb[:, jp * C:jp * C + w])
                    # spread flush + prefetch work between pairs
                    if inter:
                        inter.pop(0)()
                for th in inter:
                    th()
                # last group flushes itself per-pair above
                pending = None if gi == n_groups_ - 1 else (osb, gi)
    nc.compile()
    nc._po_pos = po_pos
    _NC_CACHE[key] = nc
    return nc


def _class_of(span_r, span_l):
    best = None
    for r, g, lopts in R_SPECS:
        if r < span_r:
            continue
        l = next((o for o in lopts if o >= span_l), None)
        if l is None:
            continue
        key = (r * l, l // g)   # patch bytes, then chunk count
        if best is None or key < best[0]:
            best = (key, (r, l))
    return best[1] if best else None


def _reference_fallback(x, rois, offset, idx):
    """Exact numpy replica of the reference for out-of-class ROIs (safety
    net; unused for the benchmark input distribution)."""
    n = len(idx)
    if n == 0:
        return np.zeros((0, C, P, P), np.float32)
    rois = rois[idx]
    offset = offset[idx]
    bidx = rois[:, 0].astype(np.int32)
    x1 = rois[:, 1] * SCALE - _f32(0.5)
    y1 = rois[:, 2] * SCALE - _f32(0.5)
    x2 = rois[:, 3] * SCALE - _f32(0.5)
    y2 = rois[:, 4] * SCALE - _f32(0.5)
    rw = np.maximum(x2 - x1, _f32(1.0))
    rh = np.maximum(y2 - y1, _f32(1.0))
    bw, bh = rw / _f32(P), rh / _f32(P)
    off = offset.reshape(n, 2, P, P)
    off_x = GAMMA * rw[:, None, None] * off[:, 0]
    off_y = GAMMA * rh[:, None, None] * off[:, 1]
    ph = np.arange(P, dtype=np.float32)
    s = (np.arange(RATIO, dtype=np.float32) + _f32(0.5)) / _f32(RATIO)
    ybase = y1[:, None, None] + ph[None, :, None] * bh[:, None, None] + off_y
    xbase = x1[:, None, None] + ph[None, None, :] * bw[:, None, None] + off_x
    ys = ybase[..., None, None] + s[:, None][None, None, None] * bh[:, None, None, None, None]
    xs = xbase[..., None, None] + s[None, :][None, None, None] * bw[:, None, None, None, None]
    ys, xs = np.broadcast_arrays(ys, xs)
    valid = (ys > -1.0) & (ys < H) & (xs > -1.0) & (xs < W)
    yc = np.clip(ys, 0.0, _f32(H - 1))
    xc = np.clip(xs, 0.0, _f32(W - 1))
    y0 = np.floor(yc).astype(np.int32)
    x0 = np.floor(xc).astype(np.int32)
    y1i = np.minimum(y0 + 1, H - 1)
    x1i = np.minimum(x0 + 1, W - 1)
    ly = (yc - y0).astype(np.float32)
    lx = (xc - x0).astype(np.float32)
    hy, hx = _f32(1.0) - ly, _f32(1.0) - lx
    b = bidx[:, None, None, None, None]
    val = ((hy * hx)[..., None] * x[b, :, y0, x0]
           + (hy * lx)[..., None] * x[b, :, y0, x1i]
           + (ly * hx)[..., None] * x[b, :, y1i, x0]
           + (ly * lx)[..., None] * x[b, :, y1i, x1i])
    val = np.where(valid[..., None], val, _f32(0.0))
    return val.mean(axis=(3, 4)).transpose(0, 3, 1, 2)


def kernel(input, rois, offset):
    input = np.asarray(input, dtype=np.float32)
    rois = np.asarray(rois, dtype=np.float32)
    offset = np.asarray(offset, dtype=np.float32)

    xt = np.ascontiguousarray(input.transpose(0, 2, 3, 1))
    if MM_DTYPE == "bf16":
        import ml_dtypes
        xt = xt.astype(ml_dtypes.bfloat16)
    elif MM_DTYPE == "pair":
        import ml_dtypes
        hi = xt.astype(ml_dtypes.bfloat16)
        lo = (xt - hi.astype(np.float32)).astype(ml_dtypes.bfloat16)
        xt = np.ascontiguousarray(
            np.stack([hi, lo], axis=3)).reshape(B, H, W, 2 * C)
    bidx, ymin, ymax, xmin, xmax, alpha_d, beta_d = _prep(rois, offset)
    n = rois.shape[0]

    # classify ROIs; build the shared slot layout
    cls = [_class_of(ymax[i] - ymin[i] + 1, xmax[i] - xmin[i] + 1)
           for i in range(n)]
    fallback_idx = [i for i in range(n) if cls[i] is None]
    by_class = {rl: [] for rl in CLASS_ORDER}
    for i, c in enumerate(cls):
        if c is not None:
            by_class[c].append(i)
    slots_per_class = {rl: -(-len(by_class[rl]) // N_CORES)
                       for rl in CLASS_ORDER}
    layout = []
    for rl in CLASS_ORDER:
        layout.extend([rl] * slots_per_class[rl])
    layout = tuple(layout)
    n_slots = len(layout)
    fo, mt_free, _ = _layout_meta(layout)

    # per-core slot assignment: class-k ROI list round-robins over cores
    slot_roi = np.full((N_CORES, n_slots), -1, np.int64)
    for rl in CLASS_ORDER:
        lst = by_class[rl]
        base = layout.index(rl) if slots_per_class[rl] else 0
        for j, ridx in enumerate(lst):
            core, slot_j = j % N_CORES, j // N_CORES
            slot_roi[core, base + slot_j] = ridx

    # build per-core inputs
    pair = MM_DTYPE == "pair"
    cpp = 2 if pair else 1
    if MM_DTYPE in ("bf16", "pair"):
        import ml_dtypes
        mt_np_dt = ml_dtypes.bfloat16
    else:
        mt_np_dt = np.float32
    mt_all = np.zeros((N_CORES, 128, mt_free), mt_np_dt)
    po_all = np.zeros((N_CORES, n_slots), np.int32)
    po_pos = _po_positions(n_slots)
    for core in range(N_CORES):
        for slot, (R, L) in enumerate(layout):
            ridx = slot_roi[core, slot]
            if ridx < 0:
                continue
            G, Q, nk = CLASS_SPECS[(R, L)]
            py0 = min(max(int(ymin[ridx]), 0), H - R)
            px0 = min(max(int(xmin[ridx]), 0), W - L)
            blk = _mt_block(alpha_d[ridx, :, py0:py0 + R],
                            beta_d[ridx, :, px0:px0 + L], R, L)
            if pair:
                import ml_dtypes
                bh = blk.astype(ml_dtypes.bfloat16)
                bl = (blk - bh.astype(np.float32)).astype(ml_dtypes.bfloat16)
                blk = np.stack(
                    [bh.reshape(Q, nk, PP), bl.reshape(Q, nk, PP)],
                    axis=2).reshape(Q, nk * 2 * PP)
            mt_all[core, 0:Q, fo[slot]:fo[slot] + nk * cpp * PP] = blk
            # po in issue order (per group: SP-ring slots first, then ACT)
            po_all[core, po_pos[slot]] = (
                ((int(bidx[ridx]) * H + py0) * W + px0) * cpp * C)

    nc = _build_kernel(layout)
    in_maps = [{"xt": xt, "mt": mt_all[c], "po": po_all[c][None, :]}
               for c in range(N_CORES)]
    kernel.last_nc = nc
    kernel.last_in_maps = in_maps
    runner = getattr(kernel, "runner", None)
    if runner is not None:
        res = runner(nc, in_maps)
    else:
        res = bass_utils.run_bass_kernel_spmd(nc, in_maps,
                                              core_ids=list(range(N_CORES)))
    kernel.last_results = res

    out = np.zeros((n, C, P, P), np.float32)
    for core in range(N_CORES):
        dev = res.results[core]["out"]     # [n_groups, PP, OUT_GROUP*C]
        for slot in range(n_slots):
            ridx = slot_roi[core, slot]
            if ridx >= 0:
                g, s = divmod(slot, OUT_GROUP)
                out[ridx] = dev[g][:, s * C:(s + 1) * C].T.reshape(C, P, P)

    if fallback_idx:
        out[fallback_idx] = _reference_fallback(input, rois, offset,
                                                np.array(fallback_idx))
    return np.ascontiguousarray(out)



# revision 23
# speedup vs baseline: 1.0424x; 1.0424x over previous
"""DCNv2 deformable ROI pooling on 8 Trainium2 NeuronCores.

Strategy: per-bin the 4x4 bilinear sample grid is separable (y outer-product
x), so each ROI's pooled output reduces to one small accumulated matmul
    out[49 bins, 256 ch] = M[49, K] @ PatchFlat[K, 256]
where K = R*L is a flattened feature-map patch window covering the ROI's
samples and M = alpha (x) beta is built from host-precomputed per-axis
interpolation weights.  ROIs (dim 0) are sharded across the 8 cores; the
channels-last feature map is replicated.

Patch windows come in size classes (R, L) in {16,24}^2 picked per ROI from
its actual sample span; per-class slot counts are chosen identically for
every core (round-robin distribution + dummy padding) so a single NEFF runs
SPMD on all 8 cores.  Per-ROI patch addresses are runtime data (register
offset DMA).

Patch chunk layout for class (R, L) with G col-groups (G*R = Q partitions,
nk = L/G chunks): partition p = r*G + s holds pixels (row r, col s*nk + k)
for chunk k, giving a plain [Q, nk*C]-destination DMA whose source is R
contiguous L*C-element row segments.
"""

import numpy as np

import concourse.bass as bass
import concourse.mybir as mybir
import concourse.tile as tile
from concourse import bacc
import concourse.bass_utils as bass_utils

B, C, H, W = 4, 256, 128, 128
N_ROIS = 512
P = 7
PP = P * P
SCALE = np.float32(0.0625)
RATIO = 4
GAMMA = np.float32(0.1)
N_CORES = 8

# Patch size classes.  G col-groups per row: partition p = r*G + s holds
# pixels (row r, col s*nk + k) for chunk k; Q = G*R partitions, nk = L/G
# chunks.  G is chosen per R to maximize partition fill (fewer, fuller
# matmul chunks); L options per R must be multiples of G.
R_SPECS = [
    (12, 8, (8, 16, 24, 32)),
    (16, 8, (8, 16, 24, 32)),
    (24, 4, (8, 12, 16, 20, 24, 28, 32)),
    (32, 4, (12, 16, 20, 24, 28, 32)),
]
CLASS_SPECS = {}
CLASS_ORDER = []
for _r, _g, _lopts in R_SPECS:
    for _l in _lopts:
        CLASS_SPECS[(_r, _l)] = (_g, _g * _r, _l // _g)
        CLASS_ORDER.append((_r, _l))
# biggest patches first: fill the DMA pipe early, drain small slots last
CLASS_ORDER.sort(key=lambda rl: -rl[0] * rl[1])

# Matmul precision mode:
#   "f32"  - exact float32 matmuls (4 cycles/row on PE)
#   "bf16" - inputs/weights rounded to bfloat16 (1 cycle/row, ~4e-3 rel err)
#   "pair" - bfloat16 hi/lo split of both operands, 3 matmuls per chunk with
#            exact bf16xbf16 products accumulated in fp32 (~1e-5 rel err,
#            3 cycles/row net)
MM_DTYPE = "bf16"

_f32 = np.float32


def _prep(rois, offset):
    """Dense per-axis interpolation weights + per-ROI sample bounds.

    Returns (bidx, ymin, ymax, xmin, xmax, alpha_d[N,PP,H], beta_d[N,PP,W]).
    """
    n = rois.shape[0]
    bidx = rois[:, 0].astype(np.int32)
    x1 = rois[:, 1] * SCALE - _f32(0.5)
    y1 = rois[:, 2] * SCALE - _f32(0.5)
    x2 = rois[:, 3] * SCALE - _f32(0.5)
    y2 = rois[:, 4] * SCALE - _f32(0.5)
    rw = np.maximum(x2 - x1, _f32(1.0))
    rh = np.maximum(y2 - y1, _f32(1.0))
    bw = rw / _f32(P)
    bh = rh / _f32(P)
    off = offset.reshape(n, 2, P, P).astype(np.float32)
    off_x = GAMMA * rw[:, None, None] * off[:, 0]
    off_y = GAMMA * rh[:, None, None] * off[:, 1]
    ph = np.arange(P, dtype=np.float32)
    s = ((np.arange(RATIO, dtype=np.float32) + _f32(0.5)) / _f32(RATIO))
    # mirror reference.py op order exactly (float32)
    ybase = y1[:, None, None] + ph[None, :, None] * bh[:, None, None] + off_y
    xbase = x1[:, None, None] + ph[None, None, :] * bw[:, None, None] + off_x
    ys = ybase[..., None] + s[None, None, None, :] * bh[:, None, None, None]
    xs = xbase[..., None] + s[None, None, None, :] * bw[:, None, None, None]
    vy = (ys > -1.0) & (ys < H)
    vx = (xs > -1.0) & (xs < W)
    yc = np.clip(ys, _f32(0.0), _f32(H - 1))
    xc = np.clip(xs, _f32(0.0), _f32(W - 1))
    y0 = np.floor(yc).astype(np.int32)
    x0 = np.floor(xc).astype(np.int32)
    y1i = np.minimum(y0 + 1, H - 1)
    x1i = np.minimum(x0 + 1, W - 1)
    ly = (yc - y0).astype(np.float32)
    lx = (xc - x0).astype(np.float32)
    hy = _f32(1.0) - ly
    hx = _f32(1.0) - lx

    npp = n * PP
    alpha_d = np.zeros((npp, H), np.float32)
    beta_d = np.zeros((npp, W), np.float32)
    rows = np.repeat(np.arange(npp), RATIO)
    inv = _f32(1.0 / RATIO)
    np.add.at(alpha_d, (rows, y0.reshape(npp, RATIO).ravel()),
              (np.where(vy, hy, 0).reshape(npp, RATIO) * inv).ravel())
    np.add.at(alpha_d, (rows, y1i.reshape(npp, RATIO).ravel()),
              (np.where(vy, ly, 0).reshape(npp, RATIO) * inv).ravel())
    np.add.at(beta_d, (rows, x0.reshape(npp, RATIO).ravel()),
              (np.where(vx, hx, 0).reshape(npp, RATIO) * inv).ravel())
    np.add.at(beta_d, (rows, x1i.reshape(npp, RATIO).ravel()),
              (np.where(vx, lx, 0).reshape(npp, RATIO) * inv).ravel())

    ymin = np.minimum(y0.reshape(n, -1).min(axis=1), 127)
    ymax = np.minimum(y1i.reshape(n, -1).max(axis=1), 127)
    xmin = np.minimum(x0.reshape(n, -1).min(axis=1), 127)
    xmax = np.minimum(x1i.reshape(n, -1).max(axis=1), 127)
    return (bidx, ymin, ymax, xmin, xmax,
            alpha_d.reshape(n, PP, H), beta_d.reshape(n, PP, W))


def _mt_block(alpha_w, beta_w, R, L):
    """[PP, R] x [PP, L] weights -> device MT block [Q, nk*PP]."""
    G, Q, nk = CLASS_SPECS[(R, L)]
    p = np.arange(Q)
    a = alpha_w[:, p // G]                        # [PP, Q]
    l_idx = (p[:, None] % G) * nk + np.arange(nk)[None, :]   # [Q, nk]
    b = beta_w[:, l_idx]                          # [PP, Q, nk]
    mt = a.T[:, None, :] * b.transpose(1, 2, 0)   # [Q, nk, PP]
    return mt.reshape(Q, nk * PP).astype(np.float32)


def _layout_meta(layout):
    """Free-dim offsets of each slot's MT block in the resident SBUF tile,
    total free size, and the <=4 load-chunk split points (slot-aligned)."""
    pair_f = 2 if MM_DTYPE == "pair" else 1
    fo = []
    f = 0
    for rl in layout:
        G, Q, nk = CLASS_SPECS[rl]
        fo.append(f)
        f += nk * PP * pair_f
    bounds = fo + [f]
    n_chunks = 4
    splits = [0]
    for j in range(1, n_chunks):
        target = f * j // n_chunks
        splits.append(min(bounds, key=lambda b_: abs(b_ - target)))
    splits.append(f)
    splits = sorted(set(splits))
    return fo, f, splits


OUT_GROUP = 16  # slots per packed output flush
PATCH_BUFS = 16
PSUM_BUFS = 8   # [PP, 2C] f32 pair tiles, one PSUM bank each
SP_NUM, SP_DEN = 8, 16   # patch DMA share on the SP ring (rest on ACT)
OFF_CHUNK = 2   # offsets per batched register load


def _sp_flags(gs):
    """Per-slot ring assignment within a group: evenly spread SP_NUM/SP_DEN
    of the slots onto the SP ring, the rest onto ACT."""
    n_sp = (gs * SP_NUM + SP_DEN - 1) // SP_DEN
    return [(j + 1) * n_sp // gs > j * n_sp // gs for j in range(gs)]


def _po_positions(n_slots):
    """slot -> index in the po tensor (group-major, SP slots before ACT)."""
    po_pos = {}
    p = 0
    for g0 in range(0, n_slots, OUT_GROUP):
        gs = min(OUT_GROUP, n_slots - g0)
        flags = _sp_flags(gs)
        for i in [g0 + j for j in range(gs) if flags[j]] + \
                 [g0 + j for j in range(gs) if not flags[j]]:
            po_pos[i] = p
            p += 1
    return po_pos


_NC_CACHE = {}


def _build_kernel(layout):
    """layout: tuple of (R, L) per slot, identical on every core."""
    key = (tuple(layout), MM_DTYPE)
    if key in _NC_CACHE:
        return _NC_CACHE[key]
    n_slots = len(layout)
    fo, mt_free, splits = _layout_meta(layout)
    pair = MM_DTYPE == "pair"
    data_dt = (mybir.dt.bfloat16 if MM_DTYPE in ("bf16", "pair")
               else mybir.dt.float32)
    mm_dt = {"f32": mybir.dt.float32, "f32r": mybir.dt.float32r,
             "bf16": mybir.dt.bfloat16, "pair": mybir.dt.bfloat16}[MM_DTYPE]
    cpp = 2 if pair else 1  # channel planes per pixel in xt / patch

    nc = bacc.Bacc("TRN2", target_bir_lowering=False, debug=False,
                   num_devices=N_CORES)
    xt_shape = [B, H, W, cpp * C] if pair else [B, H, W, C]
    xt = nc.dram_tensor("xt", xt_shape, data_dt,
                        kind="ExternalInput").ap()
    mt = nc.dram_tensor("mt", [128, mt_free], data_dt,
                        kind="ExternalInput").ap()
    po = nc.dram_tensor("po", [1, n_slots], mybir.dt.int32,
                        kind="ExternalInput").ap()
    n_groups = -(-n_slots // OUT_GROUP)
    # group-major output: out[g, b, s*C + c] holds slot g*OUT_GROUP+s
    out = nc.dram_tensor("out", [n_groups, PP, OUT_GROUP * C],
                         mybir.dt.float32, kind="ExternalOutput").ap()

    groups = [(g, min(OUT_GROUP, n_slots - g)) for g in range(0, n_slots, OUT_GROUP)]
    n_groups_ = len(groups)
    bounds = fo + [mt_free]
    cc = cpp * C
    pair_f = 2 if pair else 1

    # per-group ring split and po issue-order positions (host mirrors this)
    g_ring = []
    for g0, gs in groups:
        flags = _sp_flags(gs)
        g_ring.append(([g0 + j for j in range(gs) if flags[j]],
                       [g0 + j for j in range(gs) if not flags[j]]))
    po_pos = _po_positions(n_slots)
    gw_max = max(bounds[g0 + gs] - bounds[g0] for g0, gs in groups)

    def _max_off(i):
        R, L = layout[i]
        return (((B - 1) * H + (H - R)) * W + (W - L)) * cc

    with tile.TileContext(nc) as tc:
        with (
            tc.tile_pool(name="offp", bufs=1) as offp,
            tc.tile_pool(name="mtp", bufs=4) as mtp,
            tc.tile_pool(name="patchp", bufs=PATCH_BUFS) as patchp,
            tc.tile_pool(name="outp", bufs=3) as outp,
            tc.tile_pool(name="psump", bufs=PSUM_BUFS, space="PSUM") as psump,
        ):
            offs = offp.tile([1, n_slots], mybir.dt.int32)
            nc.sync.dma_start(offs[:, :], po[:, :])

            mt_tiles = [None] * n_groups_
            off_vals = [None] * n_groups_

            def load_mt(gi):
                g0, gs = groups[gi]
                w = bounds[g0 + gs] - bounds[g0]
                t = mtp.tile([128, gw_max], data_dt, tag="mt")
                eng = nc.sync if gi % 2 == 0 else nc.scalar
                eng.dma_start(t[:, 0:w], mt[:, bounds[g0]:bounds[g0 + gs]])
                mt_tiles[gi] = t

            def off_thunks(gi):
                """One thunk per OFF_CHUNK register batch, to be emitted
                interleaved with patch DMAs so the ~480ns/reg load latency
                never blocks a run of patch issues."""
                off_vals[gi] = {}
                thunks = []
                for eng, lst in ((mybir.EngineType.SP, g_ring[gi][0]),
                                 (mybir.EngineType.Activation, g_ring[gi][1])):
                    if not lst:
                        continue
                    base = po_pos[lst[0]]
                    for c0 in range(0, len(lst), OFF_CHUNK):
                        chunk = lst[c0:c0 + OFF_CHUNK]

                        def th(eng=eng, base=base, c0=c0, chunk=chunk, gi=gi):
                            _, vs = nc.values_load_multi_w_load_instructions(
                                offs[0:1, base + c0:base + c0 + len(chunk)],
                                engines=[eng],
                                min_val=0,
                                max_val=max(_max_off(i) for i in chunk),
                                skip_runtime_bounds_check=True)
                            off_vals[gi].update(zip(chunk, vs))
                        thunks.append(th)
                return thunks

            for th in off_thunks(0):
                th()
            load_mt(0)
            if n_groups_ > 1:
                for th in off_thunks(1):
                    th()
                load_mt(1)

            pending = None
            for gi, (g0, gs) in enumerate(groups):
                osb = outp.tile([PP, OUT_GROUP * C], mybir.dt.float32,
                                tag="osb")
                if gs < OUT_GROUP:
                    nc.vector.memset(osb[:, gs * C:], 0.0)
                mt_sb = mt_tiles[gi]
                vals = off_vals[gi]
                sp_set = set(g_ring[gi][0])
                mb = bounds[g0]
                # register-load prefetches for group gi+2 are wait-free and
                # interleave between pairs; flush/MT (which wait on compute)
                # are emitted only after all of this group's patch DMAs
                inter = off_thunks(gi + 2) if gi + 2 < n_groups_ else []
                for jp in range(0, gs, 2):
                    pr = [g0 + jp] + ([g0 + jp + 1] if jp + 1 < gs else [])
                    ps = psump.tile([PP, 2 * C], mybir.dt.float32,
                                    space="PSUM")
                    for h, i in enumerate(pr):
                        R, L = layout[i]
                        G, Q, nk = CLASS_SPECS[(R, L)]
                        patch = patchp.tile([Q, nk * cc], data_dt,
                                            tag="patch")
                        issuer = nc.sync if i in sp_set else nc.scalar
                        src = bass.AP(xt.tensor, vals[i],
                                      [[W * cc, R], [1, L * cc]])
                        issuer.dma_start(patch[:, :], src)
                        pv = ps[:, h * C:(h + 1) * C]
                        f0 = fo[i] - mb
                        if not pair:
                            for k in range(nk):
                                lhsT = mt_sb[0:Q,
                                             f0 + k * PP:f0 + (k + 1) * PP]
                                rhs = patch[:, k * C:(k + 1) * C]
                                if mm_dt != data_dt:
                                    lhsT = lhsT.bitcast(mm_dt)
                                    rhs = rhs.bitcast(mm_dt)
                                nc.tensor.matmul(
                                    pv, lhsT=lhsT, rhs=rhs,
                                    start=(k == 0), stop=(k == nk - 1))
                        else:
                            # hi/lo pair: out = Mhi@Xhi + Mlo@Xhi + Mhi@Xlo
                            for k in range(nk):
                                mhi = mt_sb[0:Q, f0 + 2 * k * PP:
                                            f0 + (2 * k + 1) * PP]
                                mlo = mt_sb[0:Q, f0 + (2 * k + 1) * PP:
                                            f0 + (2 * k + 2) * PP]
                                xhi = patch[:, 2 * k * C:(2 * k + 1) * C]
                                xlo = patch[:, (2 * k + 1) * C:(2 * k + 2) * C]
                                nc.tensor.matmul(pv, lhsT=mhi, rhs=xhi,
                                                 start=(k == 0), stop=False)
                                nc.tensor.matmul(pv, lhsT=mlo, rhs=xhi,
                                                 start=False, stop=False)
                                nc.tensor.matmul(pv, lhsT=mhi, rhs=xlo,
                                                 start=False,
                                                 stop=(k == nk - 1))
                    nc.vector.tensor_copy(
                        osb[:, jp * C:(jp + len(pr)) * C],
                        ps[:, 0:len(pr) * C])
                    if gi == n_groups_ - 1:
                        # last group: flush each pair as soon as it's
                        # copied, so the final drain is one pair, not
                        # the whole group
                        eng = nc.sync if (jp // 2) % 2 == 0 else nc.scalar
                        w = len(pr) * C if jp + 2 < gs else \
                            (OUT_GROUP - jp) * C
                        eng.dma_start(out[gi][:, jp * C:jp * C + w],
                                      osb[:, jp * C:jp * C + w])
                    # spread next-next group's register loads between pairs
                    if inter:
                        inter.pop(0)()
                for th in inter:
                    th()
                if gi + 2 < n_groups_:
                    load_mt(gi + 2)
                if pending is not None:
                    posb, pg = pending
                    nc.sync.dma_start(out[pg][0:25], posb[0:25, :])
                    nc.scalar.dma_start(out[pg][25:PP], posb[25:PP, :])
                # last group flushes itself per-pair above
                pending = None if gi == n_groups_ - 1 else (osb, gi)
    nc.compile()
    nc._po_pos = po_pos
    _NC_CACHE[key] = nc
    return nc


def _class_of(span_r, span_l):
    best = None
    for r, g, lopts in R_SPECS:
        if r < span_r:
            continue
        l = next((o for o in lopts if o >= span_l), None)
        if l is None:
            continue
        key = (r * l, l // g)   # patch bytes, then chunk count
        if best is None or key < best[0]:
            best = (key, (r, l))
    return best[1] if best else None


def _reference_fallback(x, rois, offset, idx):
    """Exact numpy replica of the reference for out-of-class ROIs (safety
    net; unused for the benchmark input distribution)."""
    n = len(idx)
    if n == 0:
        return np.zeros((0, C, P, P), np.float32)
    rois = rois[idx]
    offset = offset[idx]
    bidx = rois[:, 0].astype(np.int32)
    x1 = rois[:, 1] * SCALE - _f32(0.5)
    y1 = rois[:, 2] * SCALE - _f32(0.5)
    x2 = rois[:, 3] * SCALE - _f32(0.5)
    y2 = rois[:, 4] * SCALE - _f32(0.5)
    rw = np.maximum(x2 - x1, _f32(1.0))
    rh = np.maximum(y2 - y1, _f32(1.0))
    bw, bh = rw / _f32(P), rh / _f32(P)
    off = offset.reshape(n, 2, P, P)
    off_x = GAMMA * rw[:, None, None] * off[:, 0]
    off_y = GAMMA * rh[:, None, None] * off[:, 1]
    ph = np.arange(P, dtype=np.float32)
    s = (np.arange(RATIO, dtype=np.float32) + _f32(0.5)) / _f32(RATIO)
    ybase = y1[:, None, None] + ph[None, :, None] * bh[:, None, None] + off_y
    xbase = x1[:, None, None] + ph[None, None, :] * bw[:, None, None] + off_x
    ys = ybase[..., None, None] + s[:, None][None, None, None] * bh[:, None, None, None, None]
    xs = xbase[..., None, None] + s[None, :][None, None, None] * bw[:, None, None, None, None]
    ys, xs = np.broadcast_arrays(ys, xs)
    valid = (ys > -1.0) & (ys < H) & (xs > -1.0) & (xs < W)
    yc = np.clip(ys, 0.0, _f32(H - 1))
    xc = np.clip(xs, 0.0, _f32(W - 1))
    y0 = np.floor(yc).astype(np.int32)
    x0 = np.floor(xc).astype(np.int32)
    y1i = np.minimum(y0 + 1, H - 1)
    x1i = np.minimum(x0 + 1, W - 1)
    ly = (yc - y0).astype(np.float32)
    lx = (xc - x0).astype(np.float32)
    hy, hx = _f32(1.0) - ly, _f32(1.0) - lx
    b = bidx[:, None, None, None, None]
    val = ((hy * hx)[..., None] * x[b, :, y0, x0]
           + (hy * lx)[..., None] * x[b, :, y0, x1i]
           + (ly * hx)[..., None] * x[b, :, y1i, x0]
           + (ly * lx)[..., None] * x[b, :, y1i, x1i])
    val = np.where(valid[..., None], val, _f32(0.0))
    return val.mean(axis=(3, 4)).transpose(0, 3, 1, 2)


def kernel(input, rois, offset):
    input = np.asarray(input, dtype=np.float32)
    rois = np.asarray(rois, dtype=np.float32)
    offset = np.asarray(offset, dtype=np.float32)

    xt = np.ascontiguousarray(input.transpose(0, 2, 3, 1))
    if MM_DTYPE == "bf16":
        import ml_dtypes
        xt = xt.astype(ml_dtypes.bfloat16)
    elif MM_DTYPE == "pair":
        import ml_dtypes
        hi = xt.astype(ml_dtypes.bfloat16)
        lo = (xt - hi.astype(np.float32)).astype(ml_dtypes.bfloat16)
        xt = np.ascontiguousarray(
            np.stack([hi, lo], axis=3)).reshape(B, H, W, 2 * C)
    bidx, ymin, ymax, xmin, xmax, alpha_d, beta_d = _prep(rois, offset)
    n = rois.shape[0]

    # classify ROIs; build the shared slot layout
    cls = [_class_of(ymax[i] - ymin[i] + 1, xmax[i] - xmin[i] + 1)
           for i in range(n)]
    fallback_idx = [i for i in range(n) if cls[i] is None]
    by_class = {rl: [] for rl in CLASS_ORDER}
    for i, c in enumerate(cls):
        if c is not None:
            by_class[c].append(i)
    slots_per_class = {rl: -(-len(by_class[rl]) // N_CORES)
                       for rl in CLASS_ORDER}
    layout = []
    for rl in CLASS_ORDER:
        layout.extend([rl] * slots_per_class[rl])
    layout = tuple(layout)
    n_slots = len(layout)
    fo, mt_free, _ = _layout_meta(layout)

    # per-core slot assignment: class-k ROI list round-robins over cores
    slot_roi = np.full((N_CORES, n_slots), -1, np.int64)
    for rl in CLASS_ORDER:
        lst = by_class[rl]
        base = layout.index(rl) if slots_per_class[rl] else 0
        for j, ridx in enumerate(lst):
            core, slot_j = j % N_CORES, j // N_CORES
            slot_roi[core, base + slot_j] = ridx

    # build per-core inputs
    pair = MM_DTYPE == "pair"
    cpp = 2 if pair else 1
    if MM_DTYPE in ("bf16", "pair"):
        import ml_dtypes
        mt_np_dt = ml_dtypes.bfloat16
    else:
        mt_np_dt = np.float32
    mt_all = np.zeros((N_CORES, 128, mt_free), mt_np_dt)
    po_all = np.zeros((N_CORES, n_slots), np.int32)
    po_pos = _po_positions(n_slots)
    for core in range(N_CORES):
        for slot, (R, L) in enumerate(layout):
            ridx = slot_roi[core, slot]
            if ridx < 0:
                continue
            G, Q, nk = CLASS_SPECS[(R, L)]
            py0 = min(max(int(ymin[ridx]), 0), H - R)
            px0 = min(max(int(xmin[ridx]), 0), W - L)
            blk = _mt_block(alpha_d[ridx, :, py0:py0 + R],
                            beta_d[ridx, :, px0:px0 + L], R, L)
            if pair:
                import ml_dtypes
                bh = blk.astype(ml_dtypes.bfloat16)
                bl = (blk - bh.astype(np.float32)).astype(ml_dtypes.bfloat16)
                blk = np.stack(
                    [bh.reshape(Q, nk, PP), bl.reshape(Q, nk, PP)],
                    axis=2).reshape(Q, nk * 2 * PP)
            mt_all[core, 0:Q, fo[slot]:fo[slot] + nk * cpp * PP] = blk
            # po in issue order (per group: SP-ring slots first, then ACT)
            po_all[core, po_pos[slot]] = (
                ((int(bidx[ridx]) * H + py0) * W + px0) * cpp * C)

    nc = _build_kernel(layout)
    in_maps = [{"xt": xt, "mt": mt_all[c], "po": po_all[c][None, :]}
               for c in range(N_CORES)]
    kernel.last_nc = nc
    kernel.last_in_maps = in_maps
    runner = getattr(kernel, "runner", None)
    if runner is not None:
        res = runner(nc, in_maps)
    else:
        res = bass_utils.run_bass_kernel_spmd(nc, in_maps,
                                              core_ids=list(range(N_CORES)))
    kernel.last_results = res

    out = np.zeros((n, C, P, P), np.float32)
    for core in range(N_CORES):
        dev = res.results[core]["out"]     # [n_groups, PP, OUT_GROUP*C]
        for slot in range(n_slots):
            ridx = slot_roi[core, slot]
            if ridx >= 0:
                g, s = divmod(slot, OUT_GROUP)
                out[ridx] = dev[g][:, s * C:(s + 1) * C].T.reshape(C, P, P)

    if fallback_idx:
        out[fallback_idx] = _reference_fallback(input, rois, offset,
                                                np.array(fallback_idx))
    return np.ascontiguousarray(out)



# revision 26
# speedup vs baseline: 1.0731x; 1.0294x over previous
"""DCNv2 deformable ROI pooling on 8 Trainium2 NeuronCores.

Strategy: per-bin the 4x4 bilinear sample grid is separable (y outer-product
x), so each ROI's pooled output reduces to one small accumulated matmul
    out[49 bins, 256 ch] = M[49, K] @ PatchFlat[K, 256]
where K = R*L is a flattened feature-map patch window covering the ROI's
samples and M = alpha (x) beta is built from host-precomputed per-axis
interpolation weights.  ROIs (dim 0) are sharded across the 8 cores; the
channels-last feature map is replicated.

Patch windows come in size classes (R, L) in {16,24}^2 picked per ROI from
its actual sample span; per-class slot counts are chosen identically for
every core (round-robin distribution + dummy padding) so a single NEFF runs
SPMD on all 8 cores.  Per-ROI patch addresses are runtime data (register
offset DMA).

Patch chunk layout for class (R, L) with G col-groups (G*R = Q partitions,
nk = L/G chunks): partition p = r*G + s holds pixels (row r, col s*nk + k)
for chunk k, giving a plain [Q, nk*C]-destination DMA whose source is R
contiguous L*C-element row segments.
"""

import numpy as np

import concourse.bass as bass
import concourse.mybir as mybir
import concourse.tile as tile
from concourse import bacc
import concourse.bass_utils as bass_utils

B, C, H, W = 4, 256, 128, 128
N_ROIS = 512
P = 7
PP = P * P
SCALE = np.float32(0.0625)
RATIO = 4
GAMMA = np.float32(0.1)
N_CORES = 8

# Patch size classes.  G col-groups per row: partition p = r*G + s holds
# pixels (row r, col s*nk + k) for chunk k; Q = G*R partitions, nk = L/G
# chunks.  G is chosen per R to maximize partition fill (fewer, fuller
# matmul chunks); L options per R must be multiples of G.
R_SPECS = [
    (12, 8, (8, 16, 24, 32)),
    (16, 8, (8, 16, 24, 32)),
    (24, 4, (8, 12, 16, 20, 24, 28, 32)),
    (32, 4, (12, 16, 20, 24, 28, 32)),
]
CLASS_SPECS = {}
CLASS_ORDER = []
for _r, _g, _lopts in R_SPECS:
    for _l in _lopts:
        CLASS_SPECS[(_r, _l)] = (_g, _g * _r, _l // _g)
        CLASS_ORDER.append((_r, _l))
# biggest patches first: fill the DMA pipe early, drain small slots last
CLASS_ORDER.sort(key=lambda rl: -rl[0] * rl[1])

# Matmul precision mode:
#   "f32"  - exact float32 matmuls (4 cycles/row on PE)
#   "bf16" - inputs/weights rounded to bfloat16 (1 cycle/row, ~4e-3 rel err)
#   "pair" - bfloat16 hi/lo split of both operands, 3 matmuls per chunk with
#            exact bf16xbf16 products accumulated in fp32 (~1e-5 rel err,
#            3 cycles/row net)
MM_DTYPE = "bf16"

_f32 = np.float32


def _prep(rois, offset):
    """Dense per-axis interpolation weights + per-ROI sample bounds.

    Returns (bidx, ymin, ymax, xmin, xmax, alpha_d[N,PP,H], beta_d[N,PP,W]).
    """
    n = rois.shape[0]
    bidx = rois[:, 0].astype(np.int32)
    x1 = rois[:, 1] * SCALE - _f32(0.5)
    y1 = rois[:, 2] * SCALE - _f32(0.5)
    x2 = rois[:, 3] * SCALE - _f32(0.5)
    y2 = rois[:, 4] * SCALE - _f32(0.5)
    rw = np.maximum(x2 - x1, _f32(1.0))
    rh = np.maximum(y2 - y1, _f32(1.0))
    bw = rw / _f32(P)
    bh = rh / _f32(P)
    off = offset.reshape(n, 2, P, P).astype(np.float32)
    off_x = GAMMA * rw[:, None, None] * off[:, 0]
    off_y = GAMMA * rh[:, None, None] * off[:, 1]
    ph = np.arange(P, dtype=np.float32)
    s = ((np.arange(RATIO, dtype=np.float32) + _f32(0.5)) / _f32(RATIO))
    # mirror reference.py op order exactly (float32)
    ybase = y1[:, None, None] + ph[None, :, None] * bh[:, None, None] + off_y
    xbase = x1[:, None, None] + ph[None, None, :] * bw[:, None, None] + off_x
    ys = ybase[..., None] + s[None, None, None, :] * bh[:, None, None, None]
    xs = xbase[..., None] + s[None, None, None, :] * bw[:, None, None, None]
    vy = (ys > -1.0) & (ys < H)
    vx = (xs > -1.0) & (xs < W)
    yc = np.clip(ys, _f32(0.0), _f32(H - 1))
    xc = np.clip(xs, _f32(0.0), _f32(W - 1))
    y0 = np.floor(yc).astype(np.int32)
    x0 = np.floor(xc).astype(np.int32)
    y1i = np.minimum(y0 + 1, H - 1)
    x1i = np.minimum(x0 + 1, W - 1)
    ly = (yc - y0).astype(np.float32)
    lx = (xc - x0).astype(np.float32)
    hy = _f32(1.0) - ly
    hx = _f32(1.0) - lx

    npp = n * PP
    alpha_d = np.zeros((npp, H), np.float32)
    beta_d = np.zeros((npp, W), np.float32)
    rows = np.repeat(np.arange(npp), RATIO)
    inv = _f32(1.0 / RATIO)
    np.add.at(alpha_d, (rows, y0.reshape(npp, RATIO).ravel()),
              (np.where(vy, hy, 0).reshape(npp, RATIO) * inv).ravel())
    np.add.at(alpha_d, (rows, y1i.reshape(npp, RATIO).ravel()),
              (np.where(vy, ly, 0).reshape(npp, RATIO) * inv).ravel())
    np.add.at(beta_d, (rows, x0.reshape(npp, RATIO).ravel()),
              (np.where(vx, hx, 0).reshape(npp, RATIO) * inv).ravel())
    np.add.at(beta_d, (rows, x1i.reshape(npp, RATIO).ravel()),
              (np.where(vx, lx, 0).reshape(npp, RATIO) * inv).ravel())

    ymin = np.minimum(y0.reshape(n, -1).min(axis=1), 127)
    ymax = np.minimum(y1i.reshape(n, -1).max(axis=1), 127)
    xmin = np.minimum(x0.reshape(n, -1).min(axis=1), 127)
    xmax = np.minimum(x1i.reshape(n, -1).max(axis=1), 127)
    return (bidx, ymin, ymax, xmin, xmax,
            alpha_d.reshape(n, PP, H), beta_d.reshape(n, PP, W))


def _mt_block(alpha_w, beta_w, R, L):
    """[PP, R] x [PP, L] weights -> device MT block [Q, nk*PP]."""
    G, Q, nk = CLASS_SPECS[(R, L)]
    p = np.arange(Q)
    a = alpha_w[:, p // G]                        # [PP, Q]
    l_idx = (p[:, None] % G) * nk + np.arange(nk)[None, :]   # [Q, nk]
    b = beta_w[:, l_idx]                          # [PP, Q, nk]
    mt = a.T[:, None, :] * b.transpose(1, 2, 0)   # [Q, nk, PP]
    return mt.reshape(Q, nk * PP).astype(np.float32)


def _layout_meta(layout):
    """Free-dim offsets of each slot's MT block in the resident SBUF tile,
    total free size, and the <=4 load-chunk split points (slot-aligned)."""
    pair_f = 2 if MM_DTYPE == "pair" else 1
    fo = []
    f = 0
    for rl in layout:
        G, Q, nk = CLASS_SPECS[rl]
        fo.append(f)
        f += nk * PP * pair_f
    bounds = fo + [f]
    n_chunks = 4
    splits = [0]
    for j in range(1, n_chunks):
        target = f * j // n_chunks
        splits.append(min(bounds, key=lambda b_: abs(b_ - target)))
    splits.append(f)
    splits = sorted(set(splits))
    return fo, f, splits


OUT_GROUP = 16  # slots per packed output flush
PATCH_BUFS = 16
PSUM_BUFS = 8   # [PP, 2C] f32 pair tiles, one PSUM bank each
SP_NUM, SP_DEN = 8, 16   # patch DMA share on the SP ring (rest on ACT)
OFF_CHUNK = 4   # offsets per batched register load


def _sp_flags(gs):
    """Per-slot ring assignment within a group: evenly spread SP_NUM/SP_DEN
    of the slots onto the SP ring, the rest onto ACT."""
    n_sp = (gs * SP_NUM + SP_DEN - 1) // SP_DEN
    return [(j + 1) * n_sp // gs > j * n_sp // gs for j in range(gs)]


def _po_positions(n_slots):
    """slot -> index in the po tensor (group-major, SP slots before ACT)."""
    po_pos = {}
    p = 0
    for g0 in range(0, n_slots, OUT_GROUP):
        gs = min(OUT_GROUP, n_slots - g0)
        flags = _sp_flags(gs)
        for i in [g0 + j for j in range(gs) if flags[j]] + \
                 [g0 + j for j in range(gs) if not flags[j]]:
            po_pos[i] = p
            p += 1
    return po_pos


_NC_CACHE = {}


def _build_kernel(layout):
    """layout: tuple of (R, L) per slot, identical on every core."""
    key = (tuple(layout), MM_DTYPE)
    if key in _NC_CACHE:
        return _NC_CACHE[key]
    n_slots = len(layout)
    fo, mt_free, splits = _layout_meta(layout)
    pair = MM_DTYPE == "pair"
    data_dt = (mybir.dt.bfloat16 if MM_DTYPE in ("bf16", "pair")
               else mybir.dt.float32)
    mm_dt = {"f32": mybir.dt.float32, "f32r": mybir.dt.float32r,
             "bf16": mybir.dt.bfloat16, "pair": mybir.dt.bfloat16}[MM_DTYPE]
    cpp = 2 if pair else 1  # channel planes per pixel in xt / patch

    nc = bacc.Bacc("TRN2", target_bir_lowering=False, debug=False,
                   num_devices=N_CORES)
    xt_shape = [B, H, W, cpp * C] if pair else [B, H, W, C]
    xt = nc.dram_tensor("xt", xt_shape, data_dt,
                        kind="ExternalInput").ap()
    mt = nc.dram_tensor("mt", [128, mt_free], data_dt,
                        kind="ExternalInput").ap()
    po = nc.dram_tensor("po", [1, n_slots], mybir.dt.int32,
                        kind="ExternalInput").ap()
    n_groups = -(-n_slots // OUT_GROUP)
    # group-major output: out[g, b, s*C + c] holds slot g*OUT_GROUP+s
    out = nc.dram_tensor("out", [n_groups, PP, OUT_GROUP * C],
                         mybir.dt.float32, kind="ExternalOutput").ap()

    groups = [(g, min(OUT_GROUP, n_slots - g)) for g in range(0, n_slots, OUT_GROUP)]
    n_groups_ = len(groups)
    bounds = fo + [mt_free]
    cc = cpp * C
    pair_f = 2 if pair else 1

    # per-group ring split and po issue-order positions (host mirrors this)
    g_ring = []
    for g0, gs in groups:
        flags = _sp_flags(gs)
        g_ring.append(([g0 + j for j in range(gs) if flags[j]],
                       [g0 + j for j in range(gs) if not flags[j]]))
    po_pos = _po_positions(n_slots)
    gw_max = max(bounds[g0 + gs] - bounds[g0] for g0, gs in groups)

    def _max_off(i):
        R, L = layout[i]
        return (((B - 1) * H + (H - R)) * W + (W - L)) * cc

    with tile.TileContext(nc) as tc:
        with (
            tc.tile_pool(name="offp", bufs=1) as offp,
            tc.tile_pool(name="mtp", bufs=4) as mtp,
            tc.tile_pool(name="patchp", bufs=PATCH_BUFS) as patchp,
            tc.tile_pool(name="outp", bufs=3) as outp,
            tc.tile_pool(name="psump", bufs=PSUM_BUFS, space="PSUM") as psump,
        ):
            offs = offp.tile([1, n_slots], mybir.dt.int32)
            nc.sync.dma_start(offs[:, :], po[:, :])

            mt_tiles = [None] * n_groups_
            off_vals = [None] * n_groups_

            def load_mt(gi):
                g0, gs = groups[gi]
                w = bounds[g0 + gs] - bounds[g0]
                t = mtp.tile([128, gw_max], data_dt, tag="mt")
                eng = nc.sync if gi % 2 == 0 else nc.scalar
                eng.dma_start(t[:, 0:w], mt[:, bounds[g0]:bounds[g0 + gs]])
                mt_tiles[gi] = t

            def off_thunks(gi):
                """One thunk per OFF_CHUNK register batch, to be emitted
                interleaved with patch DMAs so the ~480ns/reg load latency
                never blocks a run of patch issues."""
                off_vals[gi] = {}
                thunks = []
                for eng, lst in ((mybir.EngineType.SP, g_ring[gi][0]),
                                 (mybir.EngineType.Activation, g_ring[gi][1])):
                    if not lst:
                        continue
                    base = po_pos[lst[0]]
                    for c0 in range(0, len(lst), OFF_CHUNK):
                        chunk = lst[c0:c0 + OFF_CHUNK]

                        def th(eng=eng, base=base, c0=c0, chunk=chunk, gi=gi):
                            _, vs = nc.values_load_multi_w_load_instructions(
                                offs[0:1, base + c0:base + c0 + len(chunk)],
                                engines=[eng],
                                min_val=0,
                                max_val=max(_max_off(i) for i in chunk),
                                skip_runtime_bounds_check=True)
                            off_vals[gi].update(zip(chunk, vs))
                        thunks.append(th)
                return thunks

            for th in off_thunks(0):
                th()
            load_mt(0)
            if n_groups_ > 1:
                for th in off_thunks(1):
                    th()
                load_mt(1)

            pending = None
            for gi, (g0, gs) in enumerate(groups):
                osb = outp.tile([PP, OUT_GROUP * C], mybir.dt.float32,
                                tag="osb")
                if gs < OUT_GROUP:
                    nc.vector.memset(osb[:, gs * C:], 0.0)
                mt_sb = mt_tiles[gi]
                vals = off_vals[gi]
                sp_set = set(g_ring[gi][0])
                mb = bounds[g0]
                for jp in range(0, gs, 2):
                    pr = [g0 + jp] + ([g0 + jp + 1] if jp + 1 < gs else [])
                    ps = psump.tile([PP, 2 * C], mybir.dt.float32,
                                    space="PSUM")
                    for h, i in enumerate(pr):
                        R, L = layout[i]
                        G, Q, nk = CLASS_SPECS[(R, L)]
                        patch = patchp.tile([Q, nk * cc], data_dt,
                                            tag="patch")
                        issuer = nc.sync if i in sp_set else nc.scalar
                        src = bass.AP(xt.tensor, vals[i],
                                      [[W * cc, R], [1, L * cc]])
                        issuer.dma_start(patch[:, :], src)
                        pv = ps[:, h * C:(h + 1) * C]
                        f0 = fo[i] - mb
                        if not pair:
                            for k in range(nk):
                                lhsT = mt_sb[0:Q,
                                             f0 + k * PP:f0 + (k + 1) * PP]
                                rhs = patch[:, k * C:(k + 1) * C]
                                if mm_dt != data_dt:
                                    lhsT = lhsT.bitcast(mm_dt)
                                    rhs = rhs.bitcast(mm_dt)
                                nc.tensor.matmul(
                                    pv, lhsT=lhsT, rhs=rhs,
                                    start=(k == 0), stop=(k == nk - 1))
                        else:
                            # hi/lo pair: out = Mhi@Xhi + Mlo@Xhi + Mhi@Xlo
                            for k in range(nk):
                                mhi = mt_sb[0:Q, f0 + 2 * k * PP:
                                            f0 + (2 * k + 1) * PP]
                                mlo = mt_sb[0:Q, f0 + (2 * k + 1) * PP:
                                            f0 + (2 * k + 2) * PP]
                                xhi = patch[:, 2 * k * C:(2 * k + 1) * C]
                                xlo = patch[:, (2 * k + 1) * C:(2 * k + 2) * C]
                                nc.tensor.matmul(pv, lhsT=mhi, rhs=xhi,
                                                 start=(k == 0), stop=False)
                                nc.tensor.matmul(pv, lhsT=mlo, rhs=xhi,
                                                 start=False, stop=False)
                                nc.tensor.matmul(pv, lhsT=mhi, rhs=xlo,
                                                 start=False,
                                                 stop=(k == nk - 1))
                    nc.vector.tensor_copy(
                        osb[:, jp * C:(jp + len(pr)) * C],
                        ps[:, 0:len(pr) * C])
                    if gi == n_groups_ - 1:
                        # last group: flush each pair as soon as it's
                        # copied, so the final drain is one pair, not
                        # the whole group
                        eng = nc.sync if (jp // 2) % 2 == 0 else nc.scalar
                        w = len(pr) * C if jp + 2 < gs else \
                            (OUT_GROUP - jp) * C
                        eng.dma_start(out[gi][:, jp * C:jp * C + w],
                                      osb[:, jp * C:jp * C + w])
                if gi + 2 < n_groups_:
                    for th in off_thunks(gi + 2):
                        th()
                    load_mt(gi + 2)
                if pending is not None:
                    posb, pg = pending
                    nc.sync.dma_start(out[pg][0:25], posb[0:25, :])
                    nc.scalar.dma_start(out[pg][25:PP], posb[25:PP, :])
                # last group flushes itself per-pair above
                pending = None if gi == n_groups_ - 1 else (osb, gi)
    nc.compile()
    nc._po_pos = po_pos
    _NC_CACHE[key] = nc
    return nc


def _class_of(span_r, span_l):
    best = None
    for r, g, lopts in R_SPECS:
        if r < span_r:
            continue
        l = next((o for o in lopts if o >= span_l), None)
        if l is None:
            continue
        key = (r * l, l // g)   # patch bytes, then chunk count
        if best is None or key < best[0]:
            best = (key, (r, l))
    return best[1] if best else None


def _reference_fallback(x, rois, offset, idx):
    """Exact numpy replica of the reference for out-of-class ROIs (safety
    net; unused for the benchmark input distribution)."""
    n = len(idx)
    if n == 0:
        return np.zeros((0, C, P, P), np.float32)
    rois = rois[idx]
    offset = offset[idx]
    bidx = rois[:, 0].astype(np.int32)
    x1 = rois[:, 1] * SCALE - _f32(0.5)
    y1 = rois[:, 2] * SCALE - _f32(0.5)
    x2 = rois[:, 3] * SCALE - _f32(0.5)
    y2 = rois[:, 4] * SCALE - _f32(0.5)
    rw = np.maximum(x2 - x1, _f32(1.0))
    rh = np.maximum(y2 - y1, _f32(1.0))
    bw, bh = rw / _f32(P), rh / _f32(P)
    off = offset.reshape(n, 2, P, P)
    off_x = GAMMA * rw[:, None, None] * off[:, 0]
    off_y = GAMMA * rh[:, None, None] * off[:, 1]
    ph = np.arange(P, dtype=np.float32)
    s = (np.arange(RATIO, dtype=np.float32) + _f32(0.5)) / _f32(RATIO)
    ybase = y1[:, None, None] + ph[None, :, None] * bh[:, None, None] + off_y
    xbase = x1[:, None, None] + ph[None, None, :] * bw[:, None, None] + off_x
    ys = ybase[..., None, None] + s[:, None][None, None, None] * bh[:, None, None, None, None]
    xs = xbase[..., None, None] + s[None, :][None, None, None] * bw[:, None, None, None, None]
    ys, xs = np.broadcast_arrays(ys, xs)
    valid = (ys > -1.0) & (ys < H) & (xs > -1.0) & (xs < W)
    yc = np.clip(ys, 0.0, _f32(H - 1))
    xc = np.clip(xs, 0.0, _f32(W - 1))
    y0 = np.floor(yc).astype(np.int32)
    x0 = np.floor(xc).astype(np.int32)
    y1i = np.minimum(y0 + 1, H - 1)
    x1i = np.minimum(x0 + 1, W - 1)
    ly = (yc - y0).astype(np.float32)
    lx = (xc - x0).astype(np.float32)
    hy, hx = _f32(1.0) - ly, _f32(1.0) - lx
    b = bidx[:, None, None, None, None]
    val = ((hy * hx)[..., None] * x[b, :, y0, x0]
           + (hy * lx)[..., None] * x[b, :, y0, x1i]
           + (ly * hx)[..., None] * x[b, :, y1i, x0]
           + (ly * lx)[..., None] * x[b, :, y1i, x1i])
    val = np.where(valid[..., None], val, _f32(0.0))
    return val.mean(axis=(3, 4)).transpose(0, 3, 1, 2)


def kernel(input, rois, offset):
    input = np.asarray(input, dtype=np.float32)
    rois = np.asarray(rois, dtype=np.float32)
    offset = np.asarray(offset, dtype=np.float32)

    xt = np.ascontiguousarray(input.transpose(0, 2, 3, 1))
    if MM_DTYPE == "bf16":
        import ml_dtypes
        xt = xt.astype(ml_dtypes.bfloat16)
    elif MM_DTYPE == "pair":
        import ml_dtypes
        hi = xt.astype(ml_dtypes.bfloat16)
        lo = (xt - hi.astype(np.float32)).astype(ml_dtypes.bfloat16)
        xt = np.ascontiguousarray(
            np.stack([hi, lo], axis=3)).reshape(B, H, W, 2 * C)
    bidx, ymin, ymax, xmin, xmax, alpha_d, beta_d = _prep(rois, offset)
    n = rois.shape[0]

    # classify ROIs; build the shared slot layout
    cls = [_class_of(ymax[i] - ymin[i] + 1, xmax[i] - xmin[i] + 1)
           for i in range(n)]
    fallback_idx = [i for i in range(n) if cls[i] is None]
    by_class = {rl: [] for rl in CLASS_ORDER}
    for i, c in enumerate(cls):
        if c is not None:
            by_class[c].append(i)
    slots_per_class = {rl: -(-len(by_class[rl]) // N_CORES)
                       for rl in CLASS_ORDER}
    layout = []
    for rl in CLASS_ORDER:
        layout.extend([rl] * slots_per_class[rl])
    layout = tuple(layout)
    n_slots = len(layout)
    fo, mt_free, _ = _layout_meta(layout)

    # per-core slot assignment: class-k ROI list round-robins over cores
    slot_roi = np.full((N_CORES, n_slots), -1, np.int64)
    for rl in CLASS_ORDER:
        lst = by_class[rl]
        base = layout.index(rl) if slots_per_class[rl] else 0
        for j, ridx in enumerate(lst):
            core, slot_j = j % N_CORES, j // N_CORES
            slot_roi[core, base + slot_j] = ridx

    # build per-core inputs
    pair = MM_DTYPE == "pair"
    cpp = 2 if pair else 1
    if MM_DTYPE in ("bf16", "pair"):
        import ml_dtypes
        mt_np_dt = ml_dtypes.bfloat16
    else:
        mt_np_dt = np.float32
    mt_all = np.zeros((N_CORES, 128, mt_free), mt_np_dt)
    po_all = np.zeros((N_CORES, n_slots), np.int32)
    po_pos = _po_positions(n_slots)
    for core in range(N_CORES):
        for slot, (R, L) in enumerate(layout):
            ridx = slot_roi[core, slot]
            if ridx < 0:
                continue
            G, Q, nk = CLASS_SPECS[(R, L)]
            py0 = min(max(int(ymin[ridx]), 0), H - R)
            px0 = min(max(int(xmin[ridx]), 0), W - L)
            blk = _mt_block(alpha_d[ridx, :, py0:py0 + R],
                            beta_d[ridx, :, px0:px0 + L], R, L)
            if pair:
                import ml_dtypes
                bh = blk.astype(ml_dtypes.bfloat16)
                bl = (blk - bh.astype(np.float32)).astype(ml_dtypes.bfloat16)
                blk = np.stack(
                    [bh.reshape(Q, nk, PP), bl.reshape(Q, nk, PP)],
                    axis=2).reshape(Q, nk * 2 * PP)
            mt_all[core, 0:Q, fo[slot]:fo[slot] + nk * cpp * PP] = blk
            # po in issue order (per group: SP-ring slots first, then ACT)
            po_all[core, po_pos[slot]] = (
                ((int(bidx[ridx]) * H + py0) * W + px0) * cpp * C)

    nc = _build_kernel(layout)
    in_maps = [{"xt": xt, "mt": mt_all[c], "po": po_all[c][None, :]}
               for c in range(N_CORES)]
    kernel.last_nc = nc
    kernel.last_in_maps = in_maps
    runner = getattr(kernel, "runner", None)
    if runner is not None:
        res = runner(nc, in_maps)
    else:
        res = bass_utils.run_bass_kernel_spmd(nc, in_maps,
                                              core_ids=list(range(N_CORES)))
    kernel.last_results = res

    out = np.zeros((n, C, P, P), np.float32)
    for core in range(N_CORES):
        dev = res.results[core]["out"]     # [n_groups, PP, OUT_GROUP*C]
        for slot in range(n_slots):
            ridx = slot_roi[core, slot]
            if ridx >= 0:
                g, s = divmod(slot, OUT_GROUP)
                out[ridx] = dev[g][:, s * C:(s + 1) * C].T.reshape(C, P, P)

    if fallback_idx:
        out[fallback_idx] = _reference_fallback(input, rois, offset,
                                                np.array(fallback_idx))
    return np.ascontiguousarray(out)



# revision 27
# speedup vs baseline: 1.1584x; 1.0795x over previous
"""DCNv2 deformable ROI pooling on 8 Trainium2 NeuronCores.

Strategy: per-bin the 4x4 bilinear sample grid is separable (y outer-product
x), so each ROI's pooled output reduces to one small accumulated matmul
    out[49 bins, 256 ch] = M[49, K] @ PatchFlat[K, 256]
where K = R*L is a flattened feature-map patch window covering the ROI's
samples and M = alpha (x) beta is built from host-precomputed per-axis
interpolation weights.  ROIs (dim 0) are sharded across the 8 cores; the
channels-last feature map is replicated.

Patch windows come in size classes (R, L) in {16,24}^2 picked per ROI from
its actual sample span; per-class slot counts are chosen identically for
every core (round-robin distribution + dummy padding) so a single NEFF runs
SPMD on all 8 cores.  Per-ROI patch addresses are runtime data (register
offset DMA).

Patch chunk layout for class (R, L) with G col-groups (G*R = Q partitions,
nk = L/G chunks): partition p = r*G + s holds pixels (row r, col s*nk + k)
for chunk k, giving a plain [Q, nk*C]-destination DMA whose source is R
contiguous L*C-element row segments.
"""

import numpy as np

import concourse.bass as bass
import concourse.mybir as mybir
import concourse.tile as tile
from concourse import bacc
import concourse.bass_utils as bass_utils

B, C, H, W = 4, 256, 128, 128
N_ROIS = 512
P = 7
PP = P * P
SCALE = np.float32(0.0625)
RATIO = 4
GAMMA = np.float32(0.1)
N_CORES = 8

# Patch size classes.  G col-groups per row: partition p = r*G + s holds
# pixels (row r, col s*nk + k) for chunk k; Q = G*R partitions, nk = L/G
# chunks.  G is chosen per R to maximize partition fill (fewer, fuller
# matmul chunks); L options per R must be multiples of G.
R_SPECS = [
    (12, 8, (8, 16, 24, 32)),
    (16, 8, (8, 16, 24, 32)),
    (24, 4, (8, 12, 16, 20, 24, 28, 32)),
    (32, 4, (12, 16, 20, 24, 28, 32)),
]
CLASS_SPECS = {}
CLASS_ORDER = []
for _r, _g, _lopts in R_SPECS:
    for _l in _lopts:
        CLASS_SPECS[(_r, _l)] = (_g, _g * _r, _l // _g)
        CLASS_ORDER.append((_r, _l))
# biggest patches first: fill the DMA pipe early, drain small slots last
CLASS_ORDER.sort(key=lambda rl: -rl[0] * rl[1])

# Matmul precision mode:
#   "f32"  - exact float32 matmuls (4 cycles/row on PE)
#   "bf16" - inputs/weights rounded to bfloat16 (1 cycle/row, ~4e-3 rel err)
#   "pair" - bfloat16 hi/lo split of both operands, 3 matmuls per chunk with
#            exact bf16xbf16 products accumulated in fp32 (~1e-5 rel err,
#            3 cycles/row net)
MM_DTYPE = "bf16"

_f32 = np.float32


def _prep(rois, offset):
    """Dense per-axis interpolation weights + per-ROI sample bounds.

    Returns (bidx, ymin, ymax, xmin, xmax, alpha_d[N,PP,H], beta_d[N,PP,W]).
    """
    n = rois.shape[0]
    bidx = rois[:, 0].astype(np.int32)
    x1 = rois[:, 1] * SCALE - _f32(0.5)
    y1 = rois[:, 2] * SCALE - _f32(0.5)
    x2 = rois[:, 3] * SCALE - _f32(0.5)
    y2 = rois[:, 4] * SCALE - _f32(0.5)
    rw = np.maximum(x2 - x1, _f32(1.0))
    rh = np.maximum(y2 - y1, _f32(1.0))
    bw = rw / _f32(P)
    bh = rh / _f32(P)
    off = offset.reshape(n, 2, P, P).astype(np.float32)
    off_x = GAMMA * rw[:, None, None] * off[:, 0]
    off_y = GAMMA * rh[:, None, None] * off[:, 1]
    ph = np.arange(P, dtype=np.float32)
    s = ((np.arange(RATIO, dtype=np.float32) + _f32(0.5)) / _f32(RATIO))
    # mirror reference.py op order exactly (float32)
    ybase = y1[:, None, None] + ph[None, :, None] * bh[:, None, None] + off_y
    xbase = x1[:, None, None] + ph[None, None, :] * bw[:, None, None] + off_x
    ys = ybase[..., None] + s[None, None, None, :] * bh[:, None, None, None]
    xs = xbase[..., None] + s[None, None, None, :] * bw[:, None, None, None]
    vy = (ys > -1.0) & (ys < H)
    vx = (xs > -1.0) & (xs < W)
    yc = np.clip(ys, _f32(0.0), _f32(H - 1))
    xc = np.clip(xs, _f32(0.0), _f32(W - 1))
    y0 = np.floor(yc).astype(np.int32)
    x0 = np.floor(xc).astype(np.int32)
    y1i = np.minimum(y0 + 1, H - 1)
    x1i = np.minimum(x0 + 1, W - 1)
    ly = (yc - y0).astype(np.float32)
    lx = (xc - x0).astype(np.float32)
    hy = _f32(1.0) - ly
    hx = _f32(1.0) - lx

    npp = n * PP
    alpha_d = np.zeros((npp, H), np.float32)
    beta_d = np.zeros((npp, W), np.float32)
    rows = np.repeat(np.arange(npp), RATIO)
    inv = _f32(1.0 / RATIO)
    np.add.at(alpha_d, (rows, y0.reshape(npp, RATIO).ravel()),
              (np.where(vy, hy, 0).reshape(npp, RATIO) * inv).ravel())
    np.add.at(alpha_d, (rows, y1i.reshape(npp, RATIO).ravel()),
              (np.where(vy, ly, 0).reshape(npp, RATIO) * inv).ravel())
    np.add.at(beta_d, (rows, x0.reshape(npp, RATIO).ravel()),
              (np.where(vx, hx, 0).reshape(npp, RATIO) * inv).ravel())
    np.add.at(beta_d, (rows, x1i.reshape(npp, RATIO).ravel()),
              (np.where(vx, lx, 0).reshape(npp, RATIO) * inv).ravel())

    ymin = np.minimum(y0.reshape(n, -1).min(axis=1), 127)
    ymax = np.minimum(y1i.reshape(n, -1).max(axis=1), 127)
    xmin = np.minimum(x0.reshape(n, -1).min(axis=1), 127)
    xmax = np.minimum(x1i.reshape(n, -1).max(axis=1), 127)
    return (bidx, ymin, ymax, xmin, xmax,
            alpha_d.reshape(n, PP, H), beta_d.reshape(n, PP, W))


def _mt_block(alpha_w, beta_w, R, L):
    """[PP, R] x [PP, L] weights -> device MT block [Q, nk*PP]."""
    G, Q, nk = CLASS_SPECS[(R, L)]
    p = np.arange(Q)
    a = alpha_w[:, p // G]                        # [PP, Q]
    l_idx = (p[:, None] % G) * nk + np.arange(nk)[None, :]   # [Q, nk]
    b = beta_w[:, l_idx]                          # [PP, Q, nk]
    mt = a.T[:, None, :] * b.transpose(1, 2, 0)   # [Q, nk, PP]
    return mt.reshape(Q, nk * PP).astype(np.float32)


def _layout_meta(layout):
    """Free-dim offsets of each slot's MT block in the resident SBUF tile,
    total free size, and the <=4 load-chunk split points (slot-aligned)."""
    pair_f = 2 if MM_DTYPE == "pair" else 1
    fo = []
    f = 0
    for rl in layout:
        G, Q, nk = CLASS_SPECS[rl]
        fo.append(f)
        f += nk * PP * pair_f
    bounds = fo + [f]
    n_chunks = 4
    splits = [0]
    for j in range(1, n_chunks):
        target = f * j // n_chunks
        splits.append(min(bounds, key=lambda b_: abs(b_ - target)))
    splits.append(f)
    splits = sorted(set(splits))
    return fo, f, splits


OUT_GROUP = 16  # slots per packed output flush
PATCH_BUFS = 12
PSUM_BUFS = 8   # [PP, 2C] f32 pair tiles, one PSUM bank each
SP_NUM, SP_DEN = 8, 16   # patch DMA share on the SP ring (rest on ACT)
OFF_CHUNK = 4   # offsets per batched register load


def _sp_flags(gs):
    """Per-slot ring assignment within a group: evenly spread SP_NUM/SP_DEN
    of the slots onto the SP ring, the rest onto ACT."""
    n_sp = (gs * SP_NUM + SP_DEN - 1) // SP_DEN
    return [(j + 1) * n_sp // gs > j * n_sp // gs for j in range(gs)]


def _po_positions(n_slots):
    """slot -> index in the po tensor (group-major, SP slots before ACT)."""
    po_pos = {}
    p = 0
    for g0 in range(0, n_slots, OUT_GROUP):
        gs = min(OUT_GROUP, n_slots - g0)
        flags = _sp_flags(gs)
        for i in [g0 + j for j in range(gs) if flags[j]] + \
                 [g0 + j for j in range(gs) if not flags[j]]:
            po_pos[i] = p
            p += 1
    return po_pos


_NC_CACHE = {}


def _build_kernel(layout):
    """layout: tuple of (R, L) per slot, identical on every core."""
    key = (tuple(layout), MM_DTYPE)
    if key in _NC_CACHE:
        return _NC_CACHE[key]
    n_slots = len(layout)
    fo, mt_free, splits = _layout_meta(layout)
    pair = MM_DTYPE == "pair"
    data_dt = (mybir.dt.bfloat16 if MM_DTYPE in ("bf16", "pair")
               else mybir.dt.float32)
    mm_dt = {"f32": mybir.dt.float32, "f32r": mybir.dt.float32r,
             "bf16": mybir.dt.bfloat16, "pair": mybir.dt.bfloat16}[MM_DTYPE]
    cpp = 2 if pair else 1  # channel planes per pixel in xt / patch

    nc = bacc.Bacc("TRN2", target_bir_lowering=False, debug=False,
                   num_devices=N_CORES)
    xt_shape = [B, H, W, cpp * C] if pair else [B, H, W, C]
    xt = nc.dram_tensor("xt", xt_shape, data_dt,
                        kind="ExternalInput").ap()
    mt = nc.dram_tensor("mt", [128, mt_free], data_dt,
                        kind="ExternalInput").ap()
    po = nc.dram_tensor("po", [1, n_slots], mybir.dt.int32,
                        kind="ExternalInput").ap()
    n_groups = -(-n_slots // OUT_GROUP)
    # group-major output: out[g, b, s*C + c] holds slot g*OUT_GROUP+s
    out = nc.dram_tensor("out", [n_groups, PP, OUT_GROUP * C],
                         mybir.dt.float32, kind="ExternalOutput").ap()

    groups = [(g, min(OUT_GROUP, n_slots - g)) for g in range(0, n_slots, OUT_GROUP)]
    n_groups_ = len(groups)
    bounds = fo + [mt_free]
    cc = cpp * C
    pair_f = 2 if pair else 1

    # per-group ring split and po issue-order positions (host mirrors this)
    g_ring = []
    for g0, gs in groups:
        flags = _sp_flags(gs)
        g_ring.append(([g0 + j for j in range(gs) if flags[j]],
                       [g0 + j for j in range(gs) if not flags[j]]))
    po_pos = _po_positions(n_slots)
    gw_max = max(bounds[g0 + gs] - bounds[g0] for g0, gs in groups)

    def _max_off(i):
        R, L = layout[i]
        return (((B - 1) * H + (H - R)) * W + (W - L)) * cc

    with tile.TileContext(nc) as tc:
        with (
            tc.tile_pool(name="offp", bufs=1) as offp,
            tc.tile_pool(name="mtp", bufs=4) as mtp,
            tc.tile_pool(name="patchp", bufs=PATCH_BUFS) as patchp,
            tc.tile_pool(name="outp", bufs=3) as outp,
            tc.tile_pool(name="psump", bufs=PSUM_BUFS, space="PSUM") as psump,
        ):
            offs = offp.tile([1, n_slots], mybir.dt.int32)
            nc.sync.dma_start(offs[:, :], po[:, :])

            mt_tiles = [None] * n_groups_
            off_vals = [None] * n_groups_

            def load_mt(gi):
                g0, gs = groups[gi]
                w = bounds[g0 + gs] - bounds[g0]
                t = mtp.tile([128, gw_max], data_dt, tag="mt")
                eng = nc.sync if gi % 2 == 0 else nc.scalar
                eng.dma_start(t[:, 0:w], mt[:, bounds[g0]:bounds[g0 + gs]])
                mt_tiles[gi] = t

            def off_thunks(gi):
                """One thunk per OFF_CHUNK register batch, to be emitted
                interleaved with patch DMAs so the ~480ns/reg load latency
                never blocks a run of patch issues."""
                off_vals[gi] = {}
                thunks = []
                for eng, lst in ((mybir.EngineType.SP, g_ring[gi][0]),
                                 (mybir.EngineType.Activation, g_ring[gi][1])):
                    if not lst:
                        continue
                    base = po_pos[lst[0]]
                    for c0 in range(0, len(lst), OFF_CHUNK):
                        chunk = lst[c0:c0 + OFF_CHUNK]

                        def th(eng=eng, base=base, c0=c0, chunk=chunk, gi=gi):
                            _, vs = nc.values_load_multi_w_load_instructions(
                                offs[0:1, base + c0:base + c0 + len(chunk)],
                                engines=[eng],
                                min_val=0,
                                max_val=max(_max_off(i) for i in chunk),
                                skip_runtime_bounds_check=True)
                            off_vals[gi].update(zip(chunk, vs))
                        thunks.append(th)
                return thunks

            for th in off_thunks(0):
                th()
            load_mt(0)
            if n_groups_ > 1:
                for th in off_thunks(1):
                    th()
                load_mt(1)

            pending = None
            for gi, (g0, gs) in enumerate(groups):
                osb = outp.tile([PP, OUT_GROUP * C], mybir.dt.float32,
                                tag="osb")
                if gs < OUT_GROUP:
                    nc.vector.memset(osb[:, gs * C:], 0.0)
                mt_sb = mt_tiles[gi]
                vals = off_vals[gi]
                sp_set = set(g_ring[gi][0])
                mb = bounds[g0]
                for jp in range(0, gs, 2):
                    pr = [g0 + jp] + ([g0 + jp + 1] if jp + 1 < gs else [])
                    ps = psump.tile([PP, 2 * C], mybir.dt.float32,
                                    space="PSUM")
                    for h, i in enumerate(pr):
                        R, L = layout[i]
                        G, Q, nk = CLASS_SPECS[(R, L)]
                        patch = patchp.tile([Q, nk * cc], data_dt,
                                            tag="patch")
                        issuer = nc.sync if i in sp_set else nc.scalar
                        src = bass.AP(xt.tensor, vals[i],
                                      [[W * cc, R], [1, L * cc]])
                        issuer.dma_start(patch[:, :], src)
                        pv = ps[:, h * C:(h + 1) * C]
                        f0 = fo[i] - mb
                        if not pair:
                            for k in range(nk):
                                lhsT = mt_sb[0:Q,
                                             f0 + k * PP:f0 + (k + 1) * PP]
                                rhs = patch[:, k * C:(k + 1) * C]
                                if mm_dt != data_dt:
                                    lhsT = lhsT.bitcast(mm_dt)
                                    rhs = rhs.bitcast(mm_dt)
                                nc.tensor.matmul(
                                    pv, lhsT=lhsT, rhs=rhs,
                                    start=(k == 0), stop=(k == nk - 1))
                        else:
                            # hi/lo pair: out = Mhi@Xhi + Mlo@Xhi + Mhi@Xlo
                            for k in range(nk):
                                mhi = mt_sb[0:Q, f0 + 2 * k * PP:
                                            f0 + (2 * k + 1) * PP]
                                mlo = mt_sb[0:Q, f0 + (2 * k + 1) * PP:
                                            f0 + (2 * k + 2) * PP]
                                xhi = patch[:, 2 * k * C:(2 * k + 1) * C]
                                xlo = patch[:, (2 * k + 1) * C:(2 * k + 2) * C]
                                nc.tensor.matmul(pv, lhsT=mhi, rhs=xhi,
                                                 start=(k == 0), stop=False)
                                nc.tensor.matmul(pv, lhsT=mlo, rhs=xhi,
                                                 start=False, stop=False)
                                nc.tensor.matmul(pv, lhsT=mhi, rhs=xlo,
                                                 start=False,
                                                 stop=(k == nk - 1))
                    nc.vector.tensor_copy(
                        osb[:, jp * C:(jp + len(pr)) * C],
                        ps[:, 0:len(pr) * C])
                    if gi == n_groups_ - 1:
                        # last group: flush each pair as soon as it's
                        # copied, so the final drain is one pair, not
                        # the whole group
                        eng = nc.sync if (jp // 2) % 2 == 0 else nc.scalar
                        w = len(pr) * C if jp + 2 < gs else \
                            (OUT_GROUP - jp) * C
                        eng.dma_start(out[gi][:, jp * C:jp * C + w],
                                      osb[:, jp * C:jp * C + w])
                if gi + 2 < n_groups_:
                    for th in off_thunks(gi + 2):
                        th()
                    load_mt(gi + 2)
                if pending is not None:
                    posb, pg = pending
                    nc.sync.dma_start(out[pg][0:25], posb[0:25, :])
                    nc.scalar.dma_start(out[pg][25:PP], posb[25:PP, :])
                # last group flushes itself per-pair above
                pending = None if gi == n_groups_ - 1 else (osb, gi)
    nc.compile()
    nc._po_pos = po_pos
    _NC_CACHE[key] = nc
    return nc


def _class_of(span_r, span_l):
    best = None
    for r, g, lopts in R_SPECS:
        if r < span_r:
            continue
        l = next((o for o in lopts if o >= span_l), None)
        if l is None:
            continue
        key = (r * l, l // g)   # patch bytes, then chunk count
        if best is None or key < best[0]:
            best = (key, (r, l))
    return best[1] if best else None


def _reference_fallback(x, rois, offset, idx):
    """Exact numpy replica of the reference for out-of-class ROIs (safety
    net; unused for the benchmark input distribution)."""
    n = len(idx)
    if n == 0:
        return np.zeros((0, C, P, P), np.float32)
    rois = rois[idx]
    offset = offset[idx]
    bidx = rois[:, 0].astype(np.int32)
    x1 = rois[:, 1] * SCALE - _f32(0.5)
    y1 = rois[:, 2] * SCALE - _f32(0.5)
    x2 = rois[:, 3] * SCALE - _f32(0.5)
    y2 = rois[:, 4] * SCALE - _f32(0.5)
    rw = np.maximum(x2 - x1, _f32(1.0))
    rh = np.maximum(y2 - y1, _f32(1.0))
    bw, bh = rw / _f32(P), rh / _f32(P)
    off = offset.reshape(n, 2, P, P)
    off_x = GAMMA * rw[:, None, None] * off[:, 0]
    off_y = GAMMA * rh[:, None, None] * off[:, 1]
    ph = np.arange(P, dtype=np.float32)
    s = (np.arange(RATIO, dtype=np.float32) + _f32(0.5)) / _f32(RATIO)
    ybase = y1[:, None, None] + ph[None, :, None] * bh[:, None, None] + off_y
    xbase = x1[:, None, None] + ph[None, None, :] * bw[:, None, None] + off_x
    ys = ybase[..., None, None] + s[:, None][None, None, None] * bh[:, None, None, None, None]
    xs = xbase[..., None, None] + s[None, :][None, None, None] * bw[:, None, None, None, None]
    ys, xs = np.broadcast_arrays(ys, xs)
    valid = (ys > -1.0) & (ys < H) & (xs > -1.0) & (xs < W)
    yc = np.clip(ys, 0.0, _f32(H - 1))
    xc = np.clip(xs, 0.0, _f32(W - 1))
    y0 = np.floor(yc).astype(np.int32)
    x0 = np.floor(xc).astype(np.int32)
    y1i = np.minimum(y0 + 1, H - 1)
    x1i = np.minimum(x0 + 1, W - 1)
    ly = (yc - y0).astype(np.float32)
    lx = (xc - x0).astype(np.float32)
    hy, hx = _f32(1.0) - ly, _f32(1.0) - lx
    b = bidx[:, None, None, None, None]
    val = ((hy * hx)[..., None] * x[b, :, y0, x0]
           + (hy * lx)[..., None] * x[b, :, y0, x1i]
           + (ly * hx)[..., None] * x[b, :, y1i, x0]
           + (ly * lx)[..., None] * x[b, :, y1i, x1i])
    val = np.where(valid[..., None], val, _f32(0.0))
    return val.mean(axis=(3, 4)).transpose(0, 3, 1, 2)


def kernel(input, rois, offset):
    input = np.asarray(input, dtype=np.float32)
    rois = np.asarray(rois, dtype=np.float32)
    offset = np.asarray(offset, dtype=np.float32)

    xt = np.ascontiguousarray(input.transpose(0, 2, 3, 1))
    if MM_DTYPE == "bf16":
        import ml_dtypes
        xt = xt.astype(ml_dtypes.bfloat16)
    elif MM_DTYPE == "pair":
        import ml_dtypes
        hi = xt.astype(ml_dtypes.bfloat16)
        lo = (xt - hi.astype(np.float32)).astype(ml_dtypes.bfloat16)
        xt = np.ascontiguousarray(
            np.stack([hi, lo], axis=3)).reshape(B, H, W, 2 * C)
    bidx, ymin, ymax, xmin, xmax, alpha_d, beta_d = _prep(rois, offset)
    n = rois.shape[0]

    # classify ROIs; build the shared slot layout
    cls = [_class_of(ymax[i] - ymin[i] + 1, xmax[i] - xmin[i] + 1)
           for i in range(n)]
    fallback_idx = [i for i in range(n) if cls[i] is None]
    by_class = {rl: [] for rl in CLASS_ORDER}
    for i, c in enumerate(cls):
        if c is not None:
            by_class[c].append(i)
    slots_per_class = {rl: -(-len(by_class[rl]) // N_CORES)
                       for rl in CLASS_ORDER}
    layout = []
    for rl in CLASS_ORDER:
        layout.extend([rl] * slots_per_class[rl])
    layout = tuple(layout)
    n_slots = len(layout)
    fo, mt_free, _ = _layout_meta(layout)

    # per-core slot assignment: class-k ROI list round-robins over cores
    slot_roi = np.full((N_CORES, n_slots), -1, np.int64)
    for rl in CLASS_ORDER:
        lst = by_class[rl]
        base = layout.index(rl) if slots_per_class[rl] else 0
        for j, ridx in enumerate(lst):
            core, slot_j = j % N_CORES, j // N_CORES
            slot_roi[core, base + slot_j] = ridx

    # build per-core inputs
    pair = MM_DTYPE == "pair"
    cpp = 2 if pair else 1
    if MM_DTYPE in ("bf16", "pair"):
        import ml_dtypes
        mt_np_dt = ml_dtypes.bfloat16
    else:
        mt_np_dt = np.float32
    mt_all = np.zeros((N_CORES, 128, mt_free), mt_np_dt)
    po_all = np.zeros((N_CORES, n_slots), np.int32)
    po_pos = _po_positions(n_slots)
    for core in range(N_CORES):
        for slot, (R, L) in enumerate(layout):
            ridx = slot_roi[core, slot]
            if ridx < 0:
                continue
            G, Q, nk = CLASS_SPECS[(R, L)]
            py0 = min(max(int(ymin[ridx]), 0), H - R)
            px0 = min(max(int(xmin[ridx]), 0), W - L)
            blk = _mt_block(alpha_d[ridx, :, py0:py0 + R],
                            beta_d[ridx, :, px0:px0 + L], R, L)
            if pair:
                import ml_dtypes
                bh = blk.astype(ml_dtypes.bfloat16)
                bl = (blk - bh.astype(np.float32)).astype(ml_dtypes.bfloat16)
                blk = np.stack(
                    [bh.reshape(Q, nk, PP), bl.reshape(Q, nk, PP)],
                    axis=2).reshape(Q, nk * 2 * PP)
            mt_all[core, 0:Q, fo[slot]:fo[slot] + nk * cpp * PP] = blk
            # po in issue order (per group: SP-ring slots first, then ACT)
            po_all[core, po_pos[slot]] = (
                ((int(bidx[ridx]) * H + py0) * W + px0) * cpp * C)

    nc = _build_kernel(layout)
    in_maps = [{"xt": xt, "mt": mt_all[c], "po": po_all[c][None, :]}
               for c in range(N_CORES)]
    kernel.last_nc = nc
    kernel.last_in_maps = in_maps
    runner = getattr(kernel, "runner", None)
    if runner is not None:
        res = runner(nc, in_maps)
    else:
        res = bass_utils.run_bass_kernel_spmd(nc, in_maps,
                                              core_ids=list(range(N_CORES)))
    kernel.last_results = res

    out = np.zeros((n, C, P, P), np.float32)
    for core in range(N_CORES):
        dev = res.results[core]["out"]     # [n_groups, PP, OUT_GROUP*C]
        for slot in range(n_slots):
            ridx = slot_roi[core, slot]
            if ridx >= 0:
                g, s = divmod(slot, OUT_GROUP)
                out[ridx] = dev[g][:, s * C:(s + 1) * C].T.reshape(C, P, P)

    if fallback_idx:
        out[fallback_idx] = _reference_fallback(input, rois, offset,
                                                np.array(fallback_idx))
    return np.ascontiguousarray(out)

